# revision 1
# baseline (speedup 1.0000x reference)
"""GQA causal attention block (B=4, S=1024, D=4096, H=32, KH=8, HD=128) on 8
Trainium2 NeuronCores.

Sharding: data-parallel over (batch, sequence-half) -> 8 independent cores, no
collectives. Each core computes the full attention output rows for its 512
query tokens (one half-sequence of one batch element), including Q/K/V
projections (K/V over the whole 1024-token sequence), RoPE, causal softmax
attention, and the output projection.

SPMD uniformity trick: the program is identical on every core; all per-core
variation (which tokens are queries, causality, RoPE angles) is carried in the
input DATA. Each core receives its batch's tokens permuted to [other-half,
own-half] order, so its query tokens always sit at positions [512:1024), and a
per-core additive mask column-permuted the same way encodes causality exactly.

RoPE trick: wq/wk columns are host-permuted within each head to [even dims,
odd dims] ("a|b" halves). Rotation then becomes rot(q) = q*cos + (S@q)*sin
with a constant 128x128 +-1 swap matrix S applied per head via one matmul
(dot products are invariant to the in-head permutation as long as q and k use
the same one; wv/wo are untouched).

Matmuls run in fp16 (same 11-bit mantissa as TF32/f32r, half the DMA bytes);
softmax statistics and normalization run in fp32. Scores are biased by -8
before exp (folded into the mask) so exp stays well inside fp16 range; the
softmax division removes the bias exactly.
"""

import numpy as np

import concourse.bass as bass
import concourse.tile as tile
from concourse import bacc, mybir
from concourse.bass_utils import run_bass_kernel_spmd

B, S, D = 4, 1024, 4096
H, KH, HD = 32, 8, 128
HALF = S // 2                    # tokens per core
N_CORES = 8
SCALE = 1.0 / float(np.sqrt(HD))
EXP_BIAS = -8.0                  # subtracted from scaled scores pre-exp
NEG = -1e9

MM_DT = mybir.dt.float16
MM_NP = np.float16
F32 = mybir.dt.float32
BF16 = mybir.dt.bfloat16

DT = D // 128                    # 32 d-tiles
QJT = H                          # 32 q-head j-tiles
KJT = KH                         # 8 kv j-tiles
TT = HALF // 128                 # 4 token-tiles per 512-chunk

_compiled = None


def _two_pi_split():
    two_pi = 2.0 * np.pi
    c1_bits = np.float32(two_pi).view(np.uint32) & np.uint32(0xFFFFF000)
    c1 = float(c1_bits.view(np.float32))        # ~12-bit mantissa head
    c2 = float(np.float64(two_pi) - np.float64(c1))
    return c1, c2


def _build():
    nc = bacc.Bacc("TRN2", target_bir_lowering=False, debug=False,
                   num_devices=N_CORES)

    x = nc.dram_tensor("x", [S, D], MM_DT, kind="ExternalInput").ap()
    wq = nc.dram_tensor("wq", [D, H * HD], MM_DT, kind="ExternalInput").ap()
    wk = nc.dram_tensor("wk", [D, KH * HD], MM_DT, kind="ExternalInput").ap()
    wv = nc.dram_tensor("wv", [D, KH * HD], MM_DT, kind="ExternalInput").ap()
    wo = nc.dram_tensor("wo", [H * HD, D], MM_DT, kind="ExternalInput").ap()
    maskT = nc.dram_tensor("maskT", [S, HALF], BF16, kind="ExternalInput").ap()
    freqsT = nc.dram_tensor("freqsT", [HD // 2, S], F32, kind="ExternalInput").ap()
    rotT_d = nc.dram_tensor("rotT", [128, 128], MM_DT, kind="ExternalInput").ap()
    ones_d = nc.dram_tensor("ones", [128, 128], MM_DT, kind="ExternalInput").ap()
    bias03_d = nc.dram_tensor("bias03", [128, 1], F32, kind="ExternalInput").ap()
    out = nc.dram_tensor("out", [HALF, D], F32, kind="ExternalOutput").ap()

    # DRAM spill buffers for projection outputs (re-streamed in attention)
    qT_d = nc.dram_tensor("qT_spill", [QJT, 128, HALF], MM_DT).ap()
    kT_d = nc.dram_tensor("kT_spill", [KJT, 128, S], MM_DT).ap()
    vT_d = nc.dram_tensor("vT_spill", [KJT, 128, S], MM_DT).ap()

    c1, c2 = _two_pi_split()
    INV_2PI = 1.0 / (2.0 * np.pi)
    PI_HALF = float(np.pi / 2)

    from contextlib import ExitStack

    es = ExitStack()
    with tile.TileContext(nc) as tc, es:
        const = es.enter_context(tc.tile_pool(name="const", bufs=1))
        trig = es.enter_context(tc.tile_pool(name="trig", bufs=1))
        trigw = es.enter_context(tc.tile_pool(name="trigw", bufs=3))
        maskp = es.enter_context(tc.tile_pool(name="maskp", bufs=1))
        big = es.enter_context(tc.tile_pool(name="big", bufs=2))
        kld = es.enter_context(tc.tile_pool(name="kld", bufs=3))
        wbuf = es.enter_context(tc.tile_pool(name="wbuf", bufs=4))
        pw = es.enter_context(tc.tile_pool(name="pw", bufs=2))
        sw = es.enter_context(tc.tile_pool(name="sw", bufs=3))
        pr = es.enter_context(tc.tile_pool(name="pr", bufs=4))
        qa = es.enter_context(tc.tile_pool(name="qa", bufs=2))
        vn = es.enter_context(tc.tile_pool(name="vn", bufs=2))
        rb = es.enter_context(tc.tile_pool(name="rb", bufs=2))
        ow = es.enter_context(tc.tile_pool(name="ow", bufs=4))
        ps_acc = es.enter_context(tc.tile_pool(name="ps_acc", bufs=5, space="PSUM"))
        ps_sc = es.enter_context(tc.tile_pool(name="ps_sc", bufs=3, space="PSUM"))

        # ---- constants ----
        rotT = const.tile([128, 128], MM_DT, tag="rot")
        nc.sync.dma_start(out=rotT, in_=rotT_d)
        ones = const.tile([128, 128], MM_DT, tag="ones")
        nc.sync.dma_start(out=ones, in_=ones_d)

        mask_t = maskp.tile([128, KJT, HALF], BF16, tag="mask")
        nc.sync.dma_start(
            out=mask_t, in_=maskT.rearrange("(t p) q -> p t q", p=128))

        # ---- cos/sin tables: [128, S]; rows 0:64 = freq f, 64:128 dup ----
        fr = trig.tile([64, S], F32, tag="fr")
        nc.sync.dma_start(out=fr, in_=freqsT)
        cosT = trig.tile([128, S], F32, tag="cos")
        sinT = trig.tile([128, S], F32, tag="sin")

        def trig_table(dst, shift, bias):
            t = trigw.tile([64, S], F32, tag="tw")
            nc.vector.tensor_scalar(out=t, in0=fr, scalar1=INV_2PI,
                                    scalar2=shift,
                                    op0=mybir.AluOpType.mult,
                                    op1=mybir.AluOpType.add)
            ni = trigw.tile([64, S], mybir.dt.int32, tag="tw")
            nc.vector.tensor_copy(ni, t)           # round-to-nearest
            nf = trigw.tile([64, S], F32, tag="tw")
            nc.vector.tensor_copy(nf, ni)
            r1 = trigw.tile([64, S], F32, tag="tw")
            nc.vector.scalar_tensor_tensor(
                out=r1, in0=nf, scalar=-c1, in1=fr,
                op0=mybir.AluOpType.mult, op1=mybir.AluOpType.add)
            r2 = trigw.tile([64, S], F32, tag="tw")
            nc.vector.scalar_tensor_tensor(
                out=r2, in0=nf, scalar=-c2, in1=r1,
                op0=mybir.AluOpType.mult, op1=mybir.AluOpType.add)
            if bias != 0.0:
                b_t = trigw.tile([64, 1], F32, tag="bias")
                nc.vector.memset(b_t, bias)
                nc.scalar.activation(dst[0:64, :], r2,
                                     mybir.ActivationFunctionType.Sin,
                                     bias=b_t)
            else:
                nc.scalar.activation(dst[0:64, :], r2,
                                     mybir.ActivationFunctionType.Sin)
            nc.sync.dma_start(out=dst[64:128, :], in_=dst[0:64, :])

        trig_table(sinT, 0.0, 0.0)
        trig_table(cosT, 0.25, PI_HALF)

        # ---- RoPE + evict helper ----
        def rope_evict(acc, cos_cols, sin_cols, dst):
            q_s = pw.tile([128, HALF], MM_DT, tag="qs")
            nc.scalar.copy(q_s, acc)
            ps2 = ps_sc.tile([128, HALF], F32, tag="sc")
            nc.tensor.matmul(ps2, rotT, q_s, start=True, stop=True)
            t1 = pw.tile([128, HALF], F32, tag="t1")
            nc.vector.tensor_mul(t1, q_s, cos_cols)
            t2 = pw.tile([128, HALF], F32, tag="t2")
            nc.vector.tensor_mul(t2, ps2, sin_cols)
            rot = pw.tile([128, HALF], MM_DT, tag="rotout")
            nc.vector.tensor_add(rot, t1, t2)
            nc.sync.dma_start(out=dst, in_=rot)

        def plain_evict(acc, dst):
            v_s = pw.tile([128, HALF], MM_DT, tag="qs")
            nc.scalar.copy(v_s, acc)
            nc.sync.dma_start(out=dst, in_=v_s)

        def transpose_x_chunk(c, xT):
            rows = slice(c * HALF, (c + 1) * HALF)
            for d in range(DT):
                nc.scalar.dma_start_transpose(
                    xT[:, d, :], x[rows, d * 128:(d + 1) * 128])

        def proj_group(w_ap, jg, xT):
            """4 j-tile outputs [128, HALF] accumulated over all of D."""
            accs = [ps_acc.tile([128, HALF], F32, tag="acc", name=f"acc{i}")
                    for i in range(4)]
            for d in range(DT):
                w_t = wbuf.tile([128, 512], MM_DT, tag="w")
                nc.sync.dma_start(
                    out=w_t, in_=w_ap[d * 128:(d + 1) * 128,
                                      jg * 512:(jg + 1) * 512])
                for jj in range(4):
                    nc.tensor.matmul(
                        accs[jj], w_t[:, jj * 128:(jj + 1) * 128],
                        xT[:, d, :], start=(d == 0), stop=(d == DT - 1))
            return accs

        # ---- phases T+P per token-chunk ----
        for c in range(2):
            tok = slice(c * HALF, (c + 1) * HALF)
            xT = big.tile([128, DT, HALF], MM_DT, tag="big")
            transpose_x_chunk(c, xT)

            for w_ap, spill, do_rope in ((wk, kT_d, True), (wv, vT_d, False)):
                for ug in range(KJT // 4):
                    accs = proj_group(w_ap, ug, xT)
                    for jj in range(4):
                        j = ug * 4 + jj
                        if do_rope:
                            rope_evict(accs[jj], cosT[:, tok], sinT[:, tok],
                                       spill[j, :, tok])
                        else:
                            plain_evict(accs[jj], spill[j, :, tok])

            if c == 1:
                for jg in range(QJT // 4):
                    accs = proj_group(wq, jg, xT)
                    for jj in range(4):
                        rope_evict(accs[jj], cosT[:, HALF:], sinT[:, HALF:],
                                   qT_d[jg * 4 + jj])

        # ---- attention ----
        # per-tile exp bias vectors (per-partition scalars for ACT):
        # bias03: per-core input (-8 if other-half keys are past, -1e9 if
        # future); m8: constant -8 (unmasked own-half region).
        bias03_t = const.tile([128, 1], F32, tag="b03")
        nc.sync.dma_start(out=bias03_t, in_=bias03_d)
        m8_t = const.tile([128, 1], F32, tag="m8")
        nc.vector.memset(m8_t, EXP_BIAS)
        neg_t = const.tile([128, 1], F32, tag="negb")
        nc.vector.memset(neg_t, NEG)

        def attend_head(h, kT_s, v_n, attnT):
            q_s = qa.tile([128, HALF], MM_DT, tag="qa")
            nc.sync.dma_start(out=q_s, in_=qT_d[h])
            oT_ps = ps_acc.tile([128, HALF], F32, tag="acc")
            sum_ps = ps_acc.tile([128, HALF], F32, tag="acc")
            ntau = S // 128
            for tau in range(ntau):
                sc_ps = ps_sc.tile([128, HALF], F32, tag="sc")
                nc.tensor.matmul(
                    sc_ps, kT_s[:, tau * 128:(tau + 1) * 128], q_s,
                    start=True, stop=True)
                p_t = pr.tile([128, HALF], MM_DT, tag="pr")
                if tau < 4:
                    # other-half keys: mask is uniform per core -> ACT bias
                    nc.scalar.activation(p_t, sc_ps,
                                         mybir.ActivationFunctionType.Exp,
                                         bias=bias03_t, scale=SCALE)
                else:
                    # own-half keys: causal mask tile (includes -8 bias)
                    sc_s = sw.tile([128, HALF], F32, tag="sw")
                    nc.vector.scalar_tensor_tensor(
                        out=sc_s, in0=sc_ps, scalar=SCALE,
                        in1=mask_t[:, tau, :],
                        op0=mybir.AluOpType.mult, op1=mybir.AluOpType.add)
                    nc.scalar.activation(p_t, sc_s,
                                         mybir.ActivationFunctionType.Exp)
                nc.tensor.matmul(oT_ps, v_n[:, tau, :], p_t, start=(tau == 0),
                                 stop=(tau == ntau - 1))
                nc.tensor.matmul(sum_ps, ones, p_t, start=(tau == 0),
                                 stop=(tau == ntau - 1))
            rB2 = rb.tile([128, HALF], F32, tag="rb2")
            nc.vector.reciprocal_approx_fast(rB2, sum_ps)
            nc.vector.tensor_mul(attnT[:, h, :], oT_ps, rB2)

        attnT = big.tile([128, H, HALF], MM_DT, tag="big")
        for kh in range(KH):
            kT_s = kld.tile([128, S], MM_DT, tag="kld")
            nc.sync.dma_start(out=kT_s, in_=kT_d[kh])
            v_n = vn.tile([128, KJT, 128], MM_DT, tag="vn")
            for tau in range(S // 128):
                nc.scalar.dma_start_transpose(
                    v_n[:, tau, :], vT_d[kh, :, tau * 128:(tau + 1) * 128])
            for qi in range(H // KH):
                attend_head(kh * (H // KH) + qi, kT_s, v_n, attnT)

        # ---- output projection ----
        def out_group(djg, attnT):
            accs = [ps_acc.tile([128, 512], F32, tag="acc", name=f"oacc{i}")
                    for i in range(TT)]
            for hd in range(H):
                w_t = wbuf.tile([128, 512], MM_DT, tag="w")
                nc.sync.dma_start(
                    out=w_t, in_=wo[hd * 128:(hd + 1) * 128,
                                    djg * 512:(djg + 1) * 512])
                for t4 in range(TT):
                    nc.tensor.matmul(
                        accs[t4], attnT[:, hd, t4 * 128:(t4 + 1) * 128],
                        w_t, start=(hd == 0), stop=(hd == H - 1))
            for t4 in range(TT):
                o_s = ow.tile([128, 512], F32, tag="ow")
                nc.any.tensor_copy(o_s, accs[t4])
                nc.sync.dma_start(
                    out=out[t4 * 128:(t4 + 1) * 128,
                            djg * 512:(djg + 1) * 512],
                    in_=o_s)

        for djg in range(D // 512):
            out_group(djg, attnT)

    nc.compile()
    return nc


def _get_compiled():
    global _compiled
    if _compiled is None:
        _compiled = _build()
    return _compiled


def _host_prep(x, freqs_cis, mask, wq, wk, wv, wo):
    """Shard + lay out inputs per core. Core c: batch c//2, seq-half c%2."""
    # in-head column permutation: [even dims, odd dims] per head
    def ab_perm(n_heads):
        p = []
        for h in range(n_heads):
            base = h * HD
            p.extend(range(base, base + HD, 2))
            p.extend(range(base + 1, base + HD, 2))
        return np.asarray(p)

    wq_p = np.ascontiguousarray(wq[:, ab_perm(H)]).astype(MM_NP)
    wk_p = np.ascontiguousarray(wk[:, ab_perm(KH)]).astype(MM_NP)
    wv_p = wv.astype(MM_NP)
    wo_p = wo.astype(MM_NP)

    # S^T for rot(q) = q*cos + (S@q)*sin with [a|b] layout:
    # S = [[0, -I],[I, 0]] (ra = -b rows, rb = a rows) -> S^T = [[0, I],[-I, 0]]
    rotT = np.zeros((128, 128), dtype=MM_NP)
    rotT[np.arange(64), np.arange(64) + 64] = 1.0
    rotT[np.arange(64) + 64, np.arange(64)] = -1.0
    ones = np.ones((128, 128), dtype=MM_NP)

    mask = np.asarray(mask, dtype=np.float32)
    freqs = np.asarray(freqs_cis, dtype=np.float32)
    import ml_dtypes

    in_maps = []
    for c in range(N_CORES):
        b, hhalf = divmod(c, 2)
        own = np.arange(hhalf * HALF, (hhalf + 1) * HALF)
        other = np.arange((1 - hhalf) * HALF, (2 - hhalf) * HALF)
        perm = np.concatenate([other, own])
        x_c = np.ascontiguousarray(x[b][perm]).astype(MM_NP)
        freqsT_c = np.ascontiguousarray(freqs[perm].T)          # [64, S]
        m = mask[own][:, perm] + np.float32(EXP_BIAS)           # [512, S]
        maskT_c = np.ascontiguousarray(m.T).astype(ml_dtypes.bfloat16)
        bias03 = np.full((128, 1),
                         NEG if hhalf == 0 else EXP_BIAS, dtype=np.float32)
        in_maps.append({
            "x": x_c, "wq": wq_p, "wk": wk_p, "wv": wv_p, "wo": wo_p,
            "maskT": maskT_c, "freqsT": freqsT_c,
            "rotT": rotT, "ones": ones, "bias03": bias03,
        })
    return in_maps


def kernel(x, freqs_cis, mask, wq, wk, wv, wo):
    nc = _get_compiled()
    in_maps = _host_prep(x, freqs_cis, mask, wq, wk, wv, wo)
    res = run_bass_kernel_spmd(nc, in_maps, list(range(N_CORES)))
    out = np.empty((B, S, D), dtype=np.float32)
    for c in range(N_CORES):
        b, hhalf = divmod(c, 2)
        out[b, hhalf * HALF:(hhalf + 1) * HALF, :] = res.results[c]["out"]
    return out



# revision 6
# speedup vs baseline: 1.4337x; 1.4337x over previous
"""GQA causal attention block (B=4, S=1024, D=4096, H=32, KH=8, HD=128) on 8
Trainium2 NeuronCores.

Sharding v2: data-parallel over (batch, q-row-parity). Each batch b is split
between cores 2b (q-row blocks [7,5,3,1], "A") and 2b+1 (blocks [6,4,2,0],
"B"), 512 query tokens per core. Each core projects Q/K/V only for its OWN
512 tokens; the pair exchanges K/V via a zero-padded pairwise AllReduce
(AllGather is broken in this runtime), so K/V projection work is not
duplicated.

Causal load balance: q-row blocks are assigned so both cores of a pair need
the same per-slot key-prefix profile C=(8,6,4,2) (in 128-row k-tiles). The
attention loop iterates original k-tile index t=7..0 with a shrinking q-col
prefix (128,128,256,256,384,384,512,512 cols). The key tiles live in the
gathered buffer at fixed positions (traversal table); causality inside the
last 128-col block of each prefix is applied by multiplying the exp'd probs
with a per-core 0/1 mask tile (data), everything else uses a uniform -8 exp
bias. This computes 20/32 of the dense score tiles per head.

RoPE trick (as baseline): wq/wk columns host-permuted per head to [even|odd]
halves; rot(q) = q*cos + (S@q)*sin with a constant 128x128 swap matrix.
cos/sin tables are precomputed on host from freqs_cis.

Matmuls in fp16; softmax statistics in fp32; exp biased by -8 (cancelled by
the normalization).
"""

import numpy as np

import concourse.bass as bass
import concourse.tile as tile
from concourse import bacc, mybir
from concourse.bass_utils import run_bass_kernel_spmd

B, S, D = 4, 1024, 4096
H, KH, HD = 32, 8, 128
HALF = S // 2                    # tokens per core
N_CORES = 8
SCALE = 1.0 / float(np.sqrt(HD))
EXP_BIAS = -8.0

MM_DT = mybir.dt.float16
MM_NP = np.float16
F32 = mybir.dt.float32

DT = D // 128                    # 32 d-tiles
GROUPS = [[0, 1], [2, 3], [4, 5], [6, 7]]

# q-row blocks per core type (slot order, need-descending)
BLOCKS_A = [7, 5, 3, 1]
BLOCKS_B = [6, 4, 2, 0]
# gathered position of original k-row t: A blocks at 0..3, B at 4..7
TRAV = [None] * 8
for _s, _r in enumerate(BLOCKS_A):
    TRAV[_r] = _s
for _s, _r in enumerate(BLOCKS_B):
    TRAV[_r] = 4 + _s
# q-col prefix width when processing original k-tile t
NCOLS = [512, 512, 384, 384, 256, 256, 128, 128]

_compiled = None


def _build():
    nc = bacc.Bacc("TRN2", target_bir_lowering=False, debug=False,
                   num_devices=N_CORES)

    xT = nc.dram_tensor("xT", [D, HALF], MM_DT, kind="ExternalInput").ap()
    wq = nc.dram_tensor("wq", [D, H * HD], MM_DT, kind="ExternalInput").ap()
    wk = nc.dram_tensor("wk", [D, KH * HD], MM_DT, kind="ExternalInput").ap()
    wv = nc.dram_tensor("wv", [D, KH * HD], MM_DT, kind="ExternalInput").ap()
    wo = nc.dram_tensor("wo", [H * HD, D], MM_DT, kind="ExternalInput").ap()
    cosT_d = nc.dram_tensor("cosT", [128, HALF], F32, kind="ExternalInput").ap()
    sinT_d = nc.dram_tensor("sinT", [128, HALF], F32, kind="ExternalInput").ap()
    dmask_d = nc.dram_tensor("dmask", [128, 8, 128], MM_DT, kind="ExternalInput").ap()
    mh0_d = nc.dram_tensor("mh0", [128, HALF], MM_DT, kind="ExternalInput").ap()
    mh1_d = nc.dram_tensor("mh1", [128, HALF], MM_DT, kind="ExternalInput").ap()
    rotT_d = nc.dram_tensor("rotT", [128, 128], MM_DT, kind="ExternalInput").ap()
    ones_d = nc.dram_tensor("ones", [128, 128], MM_DT, kind="ExternalInput").ap()
    ident_d = nc.dram_tensor("ident", [128, 128], MM_DT, kind="ExternalInput").ap()
    out = nc.dram_tensor("out", [HALF, D], F32, kind="ExternalOutput").ap()

    # pair-exchange staging: [half, 16 tiles (K 0..7 | V 8..15), 128, 512]
    stage = nc.dram_tensor("stage", [2, 16, 128, HALF], MM_DT).ap()
    gath = nc.dram_tensor("gath", [2, 16, 128, HALF], MM_DT).ap()

    from contextlib import ExitStack

    es = ExitStack()
    with tile.TileContext(nc) as tc, es:
        const = es.enter_context(tc.tile_pool(name="const", bufs=1))
        xbuf = es.enter_context(tc.tile_pool(name="xbuf", bufs=1))
        qbuf = es.enter_context(tc.tile_pool(name="qbuf", bufs=1))
        abuf = es.enter_context(tc.tile_pool(name="abuf", bufs=1))
        kbuf = es.enter_context(tc.tile_pool(name="kbuf", bufs=1))
        vbuf = es.enter_context(tc.tile_pool(name="vbuf", bufs=1))
        wbuf = es.enter_context(tc.tile_pool(name="wbuf", bufs=3))
        pw = es.enter_context(tc.tile_pool(name="pw", bufs=3))
        pr = es.enter_context(tc.tile_pool(name="pr", bufs=3))
        rb = es.enter_context(tc.tile_pool(name="rb", bufs=2))
        ow = es.enter_context(tc.tile_pool(name="ow", bufs=3))
        ps_proj = es.enter_context(tc.tile_pool(name="ps_proj", bufs=4, space="PSUM"))
        ps_attn = es.enter_context(tc.tile_pool(name="ps_attn", bufs=2, space="PSUM"))
        ps_sc = es.enter_context(tc.tile_pool(name="ps_sc", bufs=2, space="PSUM"))

        # ---- constants ----
        rotT = const.tile([128, 128], MM_DT, tag="rot")
        nc.sync.dma_start(out=rotT, in_=rotT_d)
        ones = const.tile([128, 128], MM_DT, tag="ones")
        nc.sync.dma_start(out=ones, in_=ones_d)
        ident = const.tile([128, 128], MM_DT, tag="ident")
        nc.sync.dma_start(out=ident, in_=ident_d)
        cosT = const.tile([128, HALF], F32, tag="cos")
        nc.sync.dma_start(out=cosT, in_=cosT_d)
        sinT = const.tile([128, HALF], F32, tag="sin")
        nc.sync.dma_start(out=sinT, in_=sinT_d)
        dmask = const.tile([128, 8, 128], MM_DT, tag="dmask")
        nc.sync.dma_start(out=dmask, in_=dmask_d)
        mh0 = const.tile([128, HALF], MM_DT, tag="mh0")
        nc.sync.dma_start(out=mh0, in_=mh0_d)
        mh1 = const.tile([128, HALF], MM_DT, tag="mh1")
        nc.sync.dma_start(out=mh1, in_=mh1_d)
        m8 = const.tile([128, 1], F32, tag="m8")
        nc.vector.memset(m8, EXP_BIAS)

        # ---- x: [128, 32 dtile, 512] in 8 chunks ----
        xT_s = xbuf.tile([128, DT, HALF], MM_DT, tag="x")
        for dc in range(8):
            nc.sync.dma_start(
                out=xT_s[:, dc * 4:(dc + 1) * 4, :],
                in_=xT[dc * 512:(dc + 1) * 512, :].rearrange(
                    "(t p) c -> p t c", p=128))

        def proj_group4(w_ap, jg, dma_engine):
            """Project 4 j-tiles (cols jg*512..+512 of w_ap) over all of D."""
            accs = [ps_proj.tile([128, HALF], F32, tag="acc", name=f"acc{i}")
                    for i in range(4)]
            for dp in range(16):
                w_t = wbuf.tile([128, 2, HALF], MM_DT, tag="w")
                dma_engine.dma_start(
                    out=w_t,
                    in_=w_ap[dp * 256:(dp + 1) * 256,
                             jg * 512:(jg + 1) * 512].rearrange(
                                 "(t p) j -> p t j", p=128))
                for t in range(2):
                    for jj in range(4):
                        nc.tensor.matmul(
                            accs[jj], w_t[:, t, jj * 128:(jj + 1) * 128],
                            xT_s[:, dp * 2 + t, :],
                            start=(dp == 0 and t == 0),
                            stop=(dp == 15 and t == 1))
            return accs

        def rope(acc):
            q_s = pw.tile([128, HALF], MM_DT, tag="qs")
            nc.any.tensor_copy(q_s, acc)
            ps2 = ps_sc.tile([128, HALF], F32, tag="sc")
            nc.tensor.matmul(ps2, rotT, q_s, start=True, stop=True)
            t1 = pw.tile([128, HALF], F32, tag="t1")
            nc.vector.tensor_mul(t1, q_s, cosT)
            t2 = pw.tile([128, HALF], F32, tag="t2")
            nc.vector.tensor_mul(t2, ps2, sinT)
            return t1, t2

        # ---- K/V projection (own tokens) + staging ----
        for ug in range(2):
            accs = proj_group4(wk, ug, nc.sync)
            for jj in range(4):
                j = ug * 4 + jj
                t1, t2 = rope(accs[jj])
                rot = pw.tile([128, HALF], MM_DT, tag="rot")
                nc.vector.tensor_add(rot, t1, t2)
                ks0 = pw.tile([128, HALF], MM_DT, tag="ks0")
                nc.vector.tensor_mul(ks0, rot, mh0)
                ks1 = pw.tile([128, HALF], MM_DT, tag="ks1")
                nc.vector.tensor_mul(ks1, rot, mh1)
                nc.scalar.dma_start(out=stage[0, j], in_=ks0)
                nc.scalar.dma_start(out=stage[1, j], in_=ks1)

        for ug in range(2):
            accs = proj_group4(wv, ug, nc.sync)
            for jj in range(4):
                j = ug * 4 + jj
                v_s = pw.tile([128, HALF], MM_DT, tag="qs")
                nc.any.tensor_copy(v_s, accs[jj])
                vst0 = pw.tile([128, HALF], MM_DT, tag="ks0")
                nc.vector.tensor_mul(vst0, v_s, mh0)
                vst1 = pw.tile([128, HALF], MM_DT, tag="ks1")
                nc.vector.tensor_mul(vst1, v_s, mh1)
                nc.scalar.dma_start(out=stage[0, 8 + j], in_=vst0)
                nc.scalar.dma_start(out=stage[1, 8 + j], in_=vst1)

        # ---- pair exchange (emulated AllGather) ----
        nc.gpsimd.collective_compute(
            "AllReduce", mybir.AluOpType.add, GROUPS,
            ins=[stage], outs=[gath])

        kT_s = kbuf.tile([128, KH, S], MM_DT, tag="k")
        v_n = vbuf.tile([128, KH, 8, 128], MM_DT, tag="v")
        for kvh in range(KH):
            nc.sync.dma_start(out=kT_s[:, kvh, 0:512], in_=gath[0, kvh])
            nc.sync.dma_start(out=kT_s[:, kvh, 512:1024], in_=gath[1, kvh])

        def load_v(kvh):
            # gathered V j-tile is [hd, tok]; v_n wants [tok, hd] per 128-blk
            for p in range(8):
                half, sl = (0, p) if p < 4 else (1, p - 4)
                nc.scalar.dma_start_transpose(
                    v_n[:, kvh, p, :],
                    gath[half, 8 + kvh][:, sl * 128:(sl + 1) * 128])

        # ---- Q projection + attention, interleaved per kv-group ----
        qT = qbuf.tile([128, H, HALF], MM_DT, tag="q")
        attnT = abuf.tile([128, H, HALF], MM_DT, tag="a")

        def attend(kvh, h):
            oT = ps_attn.tile([128, HALF], F32, tag="a2", name="oT")
            sum_ps = ps_attn.tile([128, HALF], F32, tag="a2", name="sum")
            for ti, t in enumerate(range(7, -1, -1)):
                cols = NCOLS[t]
                pos = TRAV[t]
                sc = ps_sc.tile([128, HALF], F32, tag="sc")
                nc.tensor.matmul(
                    sc[:, 0:cols], kT_s[:, kvh, pos * 128:(pos + 1) * 128],
                    qT[:, h, 0:cols], start=True, stop=True)
                p_t = pr.tile([128, HALF], MM_DT, tag="pr")
                nc.scalar.activation(p_t[:, 0:cols], sc[:, 0:cols],
                                     mybir.ActivationFunctionType.Exp,
                                     bias=m8, scale=SCALE)
                nc.vector.tensor_mul(p_t[:, cols - 128:cols],
                                     p_t[:, cols - 128:cols], dmask[:, t, :])
                nc.tensor.matmul(oT[:, 0:cols], v_n[:, kvh, pos, :],
                                 p_t[:, 0:cols], start=(ti == 0),
                                 stop=(ti == 7))
                nc.tensor.matmul(sum_ps[:, 0:cols], ones, p_t[:, 0:cols],
                                 start=(ti == 0), stop=(ti == 7))
            rB2 = rb.tile([128, HALF], F32, tag="rb2")
            nc.vector.reciprocal_approx_fast(rB2, sum_ps)
            nc.vector.tensor_mul(attnT[:, h, :], oT, rB2)

        for g in range(8):
            load_v(g)
            accs = proj_group4(wq, g, nc.sync)
            for jj in range(4):
                t1, t2 = rope(accs[jj])
                nc.vector.tensor_add(qT[:, g * 4 + jj, :], t1, t2)
            for hh in range(4):
                attend(g, g * 4 + hh)

        # ---- output projection ----
        for djg in range(8):
            accs = [ps_proj.tile([128, HALF], F32, tag="acc", name=f"oacc{i}")
                    for i in range(4)]
            for hp in range(16):
                wo_t = wbuf.tile([128, 2, HALF], MM_DT, tag="w")
                nc.sync.dma_start(
                    out=wo_t,
                    in_=wo[hp * 256:(hp + 1) * 256,
                           djg * 512:(djg + 1) * 512].rearrange(
                               "(t p) j -> p t j", p=128))
                for t in range(2):
                    hd = hp * 2 + t
                    for t4 in range(4):
                        nc.tensor.matmul(
                            accs[t4], attnT[:, hd, t4 * 128:(t4 + 1) * 128],
                            wo_t[:, t, :],
                            start=(hp == 0 and t == 0),
                            stop=(hp == 15 and t == 1))
            for t4 in range(4):
                o_s = ow.tile([128, HALF], F32, tag="ow")
                nc.any.tensor_copy(o_s, accs[t4])
                nc.scalar.dma_start(
                    out=out[t4 * 128:(t4 + 1) * 128,
                            djg * 512:(djg + 1) * 512],
                    in_=o_s)

    nc.compile()
    return nc


def _get_compiled():
    global _compiled
    if _compiled is None:
        _compiled = _build()
    return _compiled


def _host_prep(x, freqs_cis, mask, wq, wk, wv, wo):
    """Shard + lay out inputs per core. Core 2b+h: batch b, q-row parity h."""
    del mask  # causal structure is hardcoded in the per-core mask tiles

    def ab_perm(n_heads):
        p = []
        for hh in range(n_heads):
            base = hh * HD
            p.extend(range(base, base + HD, 2))
            p.extend(range(base + 1, base + HD, 2))
        return np.asarray(p)

    wq_p = np.ascontiguousarray(np.asarray(wq)[:, ab_perm(H)]).astype(MM_NP)
    wk_p = np.ascontiguousarray(np.asarray(wk)[:, ab_perm(KH)]).astype(MM_NP)
    wv_p = np.asarray(wv).astype(MM_NP)
    wo_p = np.asarray(wo).astype(MM_NP)

    rotT = np.zeros((128, 128), dtype=MM_NP)
    rotT[np.arange(64), np.arange(64) + 64] = 1.0
    rotT[np.arange(64) + 64, np.arange(64)] = -1.0
    ones = np.ones((128, 128), dtype=MM_NP)
    ident = np.eye(128, dtype=MM_NP)

    x = np.asarray(x, dtype=np.float32)
    freqs = np.asarray(freqs_cis, dtype=np.float64)
    tri = (np.arange(128)[:, None] <= np.arange(128)[None, :])  # k_i <= q_j

    in_maps = []
    for c in range(N_CORES):
        b, h = divmod(c, 2)
        blocks = BLOCKS_A if h == 0 else BLOCKS_B
        perm = np.concatenate(
            [np.arange(r * 128, (r + 1) * 128) for r in blocks])
        xT_c = np.ascontiguousarray(x[b][perm].T).astype(MM_NP)
        f = freqs[perm]                        # [512, 64] angles
        cos_h = np.cos(f).T.astype(np.float32)  # [64, 512]
        sin_h = np.sin(f).T.astype(np.float32)
        cosT_c = np.ascontiguousarray(np.concatenate([cos_h, cos_h], axis=0))
        sinT_c = np.ascontiguousarray(np.concatenate([sin_h, sin_h], axis=0))
        # diag masks: A: odd t -> tri, even -> ones; B: even t -> tri, odd -> 0
        dmask_c = np.empty((128, 8, 128), dtype=MM_NP)
        for t in range(8):
            if h == 0:
                dmask_c[:, t, :] = tri if (t % 2 == 1) else 1.0
            else:
                dmask_c[:, t, :] = tri if (t % 2 == 0) else 0.0
        mh0_c = np.full((128, HALF), 1.0 - h, dtype=MM_NP)
        mh1_c = np.full((128, HALF), float(h), dtype=MM_NP)
        in_maps.append({
            "xT": xT_c, "wq": wq_p, "wk": wk_p, "wv": wv_p, "wo": wo_p,
            "cosT": cosT_c, "sinT": sinT_c, "dmask": dmask_c,
            "mh0": mh0_c, "mh1": mh1_c,
            "rotT": rotT, "ones": ones, "ident": ident,
        })
    return in_maps


def kernel(x, freqs_cis, mask, wq, wk, wv, wo):
    nc = _get_compiled()
    in_maps = _host_prep(x, freqs_cis, mask, wq, wk, wv, wo)
    res = run_bass_kernel_spmd(nc, in_maps, list(range(N_CORES)))
    out = np.empty((B, S, D), dtype=np.float32)
    for c in range(N_CORES):
        b, h = divmod(c, 2)
        blocks = BLOCKS_A if h == 0 else BLOCKS_B
        perm = np.concatenate(
            [np.arange(r * 128, (r + 1) * 128) for r in blocks])
        out[b, perm, :] = res.results[c]["out"]
    return out


# revision 7
# speedup vs baseline: 1.4486x; 1.0104x over previous
"""GQA causal attention block (B=4, S=1024, D=4096, H=32, KH=8, HD=128) on 8
Trainium2 NeuronCores.

Sharding v3: data-parallel over (batch, q-row-parity). Each batch b is split
between cores 2b (q-row blocks [7,5,3,1], "A") and 2b+1 (blocks [6,4,2,0],
"B"), 512 query tokens per core. Each core projects Q/K/V only for its OWN
512 tokens; the pair exchanges K and V via zero-padded pairwise AllReduces
(AllGather is broken in this runtime), so K/V projection work is not
duplicated. V is projected directly in transposed [tok, vdim] layout (x tile
as the stationary matmul operand), so no on-chip transposes are needed.

Causal load balance: q-row blocks are assigned so both cores of a pair need
the same per-slot key-prefix profile C=(8,6,4,2) (in 128-row k-tiles). The
attention loop iterates original k-tile index t=7..0 with a growing q-col
prefix (128,128,256,256,384,384,512,512 cols). The key tiles live in the
gathered buffer at fixed positions (traversal table); causality inside the
last 128-col block of each prefix is applied by multiplying the exp'd probs
with a per-core 0/1 mask tile (data), everything else uses a uniform -8 exp
bias. This computes 20/32 of the dense score tiles per head.

The output projection runs as two passes of 16 heads (the second overlaps
the attention tail); the two partial outputs are summed on the host.

RoPE trick (as baseline): wq/wk columns host-permuted per head to [even|odd]
halves; rot(q) = q*cos + (S@q)*sin with a constant 128x128 swap matrix.
cos/sin tables are precomputed on host from freqs_cis.

Matmuls in fp16; softmax statistics in fp32; exp biased by -8 (cancelled by
the normalization).
"""

import numpy as np

import concourse.bass as bass
import concourse.tile as tile
from concourse import bacc, mybir
from concourse.bass_utils import run_bass_kernel_spmd

B, S, D = 4, 1024, 4096
H, KH, HD = 32, 8, 128
HALF = S // 2                    # tokens per core
N_CORES = 8
SCALE = 1.0 / float(np.sqrt(HD))
EXP_BIAS = -8.0

MM_DT = mybir.dt.float16
MM_NP = np.float16
F32 = mybir.dt.float32

DT = D // 128                    # 32 d-tiles
GROUPS = [[0, 1], [2, 3], [4, 5], [6, 7]]

# q-row blocks per core type (slot order, need-descending)
BLOCKS_A = [7, 5, 3, 1]
BLOCKS_B = [6, 4, 2, 0]
# gathered position of original k-row t: A blocks at 0..3, B at 4..7
TRAV = [None] * 8
for _s, _r in enumerate(BLOCKS_A):
    TRAV[_r] = _s
for _s, _r in enumerate(BLOCKS_B):
    TRAV[_r] = 4 + _s
# q-col prefix width when processing original k-tile t
NCOLS = [512, 512, 384, 384, 256, 256, 128, 128]

_compiled = None


def _build():
    nc = bacc.Bacc("TRN2", target_bir_lowering=False, debug=False,
                   num_devices=N_CORES)

    xT = nc.dram_tensor("xT", [D, HALF], MM_DT, kind="ExternalInput").ap()
    wq = nc.dram_tensor("wq", [D, H * HD], MM_DT, kind="ExternalInput").ap()
    wk = nc.dram_tensor("wk", [D, KH * HD], MM_DT, kind="ExternalInput").ap()
    wv = nc.dram_tensor("wv", [D, KH * HD], MM_DT, kind="ExternalInput").ap()
    wo = nc.dram_tensor("wo", [H * HD, D], MM_DT, kind="ExternalInput").ap()
    cosT_d = nc.dram_tensor("cosT", [128, HALF], F32, kind="ExternalInput").ap()
    sinT_d = nc.dram_tensor("sinT", [128, HALF], F32, kind="ExternalInput").ap()
    dmask_d = nc.dram_tensor("dmask", [128, 8, 128], MM_DT, kind="ExternalInput").ap()
    mh0_d = nc.dram_tensor("mh0", [128, HALF], MM_DT, kind="ExternalInput").ap()
    mh1_d = nc.dram_tensor("mh1", [128, HALF], MM_DT, kind="ExternalInput").ap()
    rotT_d = nc.dram_tensor("rotT", [128, 128], MM_DT, kind="ExternalInput").ap()
    ones_d = nc.dram_tensor("ones", [128, 128], MM_DT, kind="ExternalInput").ap()
    out0 = nc.dram_tensor("out0", [HALF, D], F32, kind="ExternalOutput").ap()
    out1 = nc.dram_tensor("out1", [HALF, D], F32, kind="ExternalOutput").ap()

    # pair-exchange staging (K: [hd, tok] j-tiles; V: [tok, vd] tiles)
    stageK = nc.dram_tensor("stageK", [2, 8, 128, HALF], MM_DT).ap()
    gathK = nc.dram_tensor("gathK", [2, 8, 128, HALF], MM_DT).ap()
    stageV = nc.dram_tensor("stageV", [2, 8, 128, HALF], MM_DT).ap()
    gathV = nc.dram_tensor("gathV", [2, 8, 128, HALF], MM_DT).ap()

    from contextlib import ExitStack

    es = ExitStack()
    with tile.TileContext(nc) as tc, es:
        const = es.enter_context(tc.tile_pool(name="const", bufs=1))
        xbuf = es.enter_context(tc.tile_pool(name="xbuf", bufs=1))
        qbuf = es.enter_context(tc.tile_pool(name="qbuf", bufs=1))
        abuf = es.enter_context(tc.tile_pool(name="abuf", bufs=1))
        kbuf = es.enter_context(tc.tile_pool(name="kbuf", bufs=1))
        vbuf = es.enter_context(tc.tile_pool(name="vbuf", bufs=1))
        wbuf = es.enter_context(tc.tile_pool(name="wbuf", bufs=3))
        pw = es.enter_context(tc.tile_pool(name="pw", bufs=3))
        pr = es.enter_context(tc.tile_pool(name="pr", bufs=3))
        rb = es.enter_context(tc.tile_pool(name="rb", bufs=2))
        ow = es.enter_context(tc.tile_pool(name="ow", bufs=3))
        ps_proj = es.enter_context(tc.tile_pool(name="ps_proj", bufs=4, space="PSUM"))
        ps_attn = es.enter_context(tc.tile_pool(name="ps_attn", bufs=2, space="PSUM"))
        ps_sc = es.enter_context(tc.tile_pool(name="ps_sc", bufs=2, space="PSUM"))

        # ---- x first (scalar queue; weights go on sync in parallel) ----
        xT_s = xbuf.tile([128, DT, HALF], MM_DT, tag="x")
        for dc in range(8):
            nc.scalar.dma_start(
                out=xT_s[:, dc * 4:(dc + 1) * 4, :],
                in_=xT[dc * 512:(dc + 1) * 512, :].rearrange(
                    "(t p) c -> p t c", p=128))

        # ---- constants (scalar queue) ----
        rotT = const.tile([128, 128], MM_DT, tag="rot")
        nc.scalar.dma_start(out=rotT, in_=rotT_d)
        ones = const.tile([128, 128], MM_DT, tag="ones")
        nc.scalar.dma_start(out=ones, in_=ones_d)
        cosT = const.tile([128, HALF], F32, tag="cos")
        nc.scalar.dma_start(out=cosT, in_=cosT_d)
        sinT = const.tile([128, HALF], F32, tag="sin")
        nc.scalar.dma_start(out=sinT, in_=sinT_d)
        dmask = const.tile([128, 8, 128], MM_DT, tag="dmask")
        nc.scalar.dma_start(out=dmask, in_=dmask_d)
        mh0 = const.tile([128, HALF], MM_DT, tag="mh0")
        nc.scalar.dma_start(out=mh0, in_=mh0_d)
        mh1 = const.tile([128, HALF], MM_DT, tag="mh1")
        nc.scalar.dma_start(out=mh1, in_=mh1_d)
        m8 = const.tile([128, 1], F32, tag="m8")
        nc.vector.memset(m8, EXP_BIAS)

        def proj_group4(w_ap, jg):
            """Project 4 j-tiles (cols jg*512..+512 of w_ap) over all of D."""
            accs = [ps_proj.tile([128, HALF], F32, tag="acc", name=f"acc{i}")
                    for i in range(4)]
            for dp in range(16):
                w_t = wbuf.tile([128, 2, HALF], MM_DT, tag="w")
                nc.sync.dma_start(
                    out=w_t,
                    in_=w_ap[dp * 256:(dp + 1) * 256,
                             jg * 512:(jg + 1) * 512].rearrange(
                                 "(t p) j -> p t j", p=128))
                for t in range(2):
                    for jj in range(4):
                        nc.tensor.matmul(
                            accs[jj], w_t[:, t, jj * 128:(jj + 1) * 128],
                            xT_s[:, dp * 2 + t, :],
                            start=(dp == 0 and t == 0),
                            stop=(dp == 15 and t == 1))
            return accs

        def projT_group4(w_ap, vg):
            """Project 4 token-tiles of (x @ w[:, vg*512:+512]) TRANSPOSED:
            accs[t4] = [128 tok, 512 vdim], x tile as stationary operand."""
            accs = [ps_proj.tile([128, HALF], F32, tag="acc", name=f"vacc{i}")
                    for i in range(4)]
            for dp in range(16):
                w_t = wbuf.tile([128, 2, HALF], MM_DT, tag="w")
                nc.sync.dma_start(
                    out=w_t,
                    in_=w_ap[dp * 256:(dp + 1) * 256,
                             vg * 512:(vg + 1) * 512].rearrange(
                                 "(t p) j -> p t j", p=128))
                for t in range(2):
                    for t4 in range(4):
                        nc.tensor.matmul(
                            accs[t4],
                            xT_s[:, dp * 2 + t, t4 * 128:(t4 + 1) * 128],
                            w_t[:, t, :],
                            start=(dp == 0 and t == 0),
                            stop=(dp == 15 and t == 1))
            return accs

        def rope(acc):
            q_s = pw.tile([128, HALF], MM_DT, tag="qs")
            nc.any.tensor_copy(q_s, acc)
            ps2 = ps_proj.tile([128, HALF], F32, tag="acc", name="rot")
            nc.tensor.matmul(ps2, rotT, q_s, start=True, stop=True)
            t1 = pw.tile([128, HALF], F32, tag="t1")
            nc.vector.tensor_mul(t1, q_s, cosT)
            t2 = pw.tile([128, HALF], F32, tag="t2")
            nc.vector.tensor_mul(t2, ps2, sinT)
            return t1, t2

        def stage_pair(src, dst0, dst1):
            s0 = pw.tile([128, HALF], MM_DT, tag="ks0")
            nc.vector.tensor_mul(s0, src, mh0)
            s1 = pw.tile([128, HALF], MM_DT, tag="ks1")
            nc.vector.tensor_mul(s1, src, mh1)
            nc.scalar.dma_start(out=dst0, in_=s0)
            nc.scalar.dma_start(out=dst1, in_=s1)

        # ---- K projection (own tokens) + rope + staging + exchange ----
        for ug in range(2):
            accs = proj_group4(wk, ug)
            for jj in range(4):
                j = ug * 4 + jj
                t1, t2 = rope(accs[jj])
                rot = pw.tile([128, HALF], MM_DT, tag="rot")
                nc.vector.tensor_add(rot, t1, t2)
                stage_pair(rot, stageK[0, j], stageK[1, j])
        nc.gpsimd.collective_compute(
            "AllReduce", mybir.AluOpType.add, GROUPS,
            ins=[stageK], outs=[gathK])
        kT_s = kbuf.tile([128, KH, S], MM_DT, tag="k")
        for kvh in range(KH):
            nc.sync.dma_start(out=kT_s[:, kvh, 0:512], in_=gathK[0, kvh])
            nc.sync.dma_start(out=kT_s[:, kvh, 512:1024], in_=gathK[1, kvh])

        # ---- V projection (transposed layout) + staging + exchange ----
        for vg in range(2):
            accs = projT_group4(wv, vg)
            for t4 in range(4):
                v_s = pw.tile([128, HALF], MM_DT, tag="qs")
                nc.any.tensor_copy(v_s, accs[t4])
                idx = t4 * 2 + vg
                stage_pair(v_s, stageV[0, idx], stageV[1, idx])
        nc.gpsimd.collective_compute(
            "AllReduce", mybir.AluOpType.add, GROUPS,
            ins=[stageV], outs=[gathV])
        v_g = vbuf.tile([128, 2, 8, HALF], MM_DT, tag="v")
        for half in range(2):
            for idx in range(8):
                nc.sync.dma_start(out=v_g[:, half, idx, :],
                                  in_=gathV[half, idx])

        def v_ap(kvh, p):
            # [128 tok, 128 vd] tile of kv-head kvh at gathered position p
            return v_g[:, p // 4, (p % 4) * 2 + (kvh // 4),
                       (kvh % 4) * 128:(kvh % 4 + 1) * 128]

        # ---- Q projection + attention, interleaved per kv-group ----
        qT = qbuf.tile([128, H, HALF], MM_DT, tag="q")
        attnT = abuf.tile([128, H, HALF], MM_DT, tag="a")

        def attend(kvh, h):
            oT = ps_attn.tile([128, HALF], F32, tag="a2", name="oT")
            sum_ps = ps_attn.tile([128, HALF], F32, tag="a2", name="sum")
            for ti, t in enumerate(range(7, -1, -1)):
                cols = NCOLS[t]
                pos = TRAV[t]
                sc = ps_sc.tile([128, HALF], F32, tag="sc")
                nc.tensor.matmul(
                    sc[:, 0:cols], kT_s[:, kvh, pos * 128:(pos + 1) * 128],
                    qT[:, h, 0:cols], start=True, stop=True)
                p_t = pr.tile([128, HALF], MM_DT, tag="pr")
                nc.scalar.activation(p_t[:, 0:cols], sc[:, 0:cols],
                                     mybir.ActivationFunctionType.Exp,
                                     bias=m8, scale=SCALE)
                nc.vector.tensor_mul(p_t[:, cols - 128:cols],
                                     p_t[:, cols - 128:cols], dmask[:, t, :])
                nc.tensor.matmul(oT[:, 0:cols], v_ap(kvh, pos),
                                 p_t[:, 0:cols], start=(ti == 0),
                                 stop=(ti == 7))
                nc.tensor.matmul(sum_ps[:, 0:cols], ones, p_t[:, 0:cols],
                                 start=(ti == 0), stop=(ti == 7))
            rB2 = rb.tile([128, HALF], F32, tag="rb2")
            nc.vector.reciprocal_approx_fast(rB2, sum_ps)
            nc.vector.tensor_mul(attnT[:, h, :], oT, rB2)

        for g in range(8):
            accs = proj_group4(wq, g)
            for jj in range(4):
                t1, t2 = rope(accs[jj])
                nc.vector.tensor_add(qT[:, g * 4 + jj, :], t1, t2)
            for hh in range(4):
                attend(g, g * 4 + hh)

        # ---- output projection: two passes of 16 heads ----
        def out_pass(p, dst):
            for djg in range(8):
                accs = [ps_proj.tile([128, HALF], F32, tag="acc",
                                     name=f"oacc{i}") for i in range(4)]
                for hp in range(8):
                    hdp = p * 8 + hp
                    wo_t = wbuf.tile([128, 2, HALF], MM_DT, tag="w")
                    nc.sync.dma_start(
                        out=wo_t,
                        in_=wo[hdp * 256:(hdp + 1) * 256,
                               djg * 512:(djg + 1) * 512].rearrange(
                                   "(t p) j -> p t j", p=128))
                    for t in range(2):
                        hd = hdp * 2 + t
                        for t4 in range(4):
                            nc.tensor.matmul(
                                accs[t4],
                                attnT[:, hd, t4 * 128:(t4 + 1) * 128],
                                wo_t[:, t, :],
                                start=(hp == 0 and t == 0),
                                stop=(hp == 7 and t == 1))
                for t4 in range(4):
                    o_s = ow.tile([128, HALF], F32, tag="ow")
                    nc.any.tensor_copy(o_s, accs[t4])
                    nc.scalar.dma_start(
                        out=dst[t4 * 128:(t4 + 1) * 128,
                                djg * 512:(djg + 1) * 512],
                        in_=o_s)

        out_pass(0, out0)
        out_pass(1, out1)

    nc.compile()
    return nc


def _get_compiled():
    global _compiled
    if _compiled is None:
        _compiled = _build()
    return _compiled


def _host_prep(x, freqs_cis, mask, wq, wk, wv, wo):
    """Shard + lay out inputs per core. Core 2b+h: batch b, q-row parity h."""
    del mask  # causal structure is hardcoded in the per-core mask tiles

    def ab_perm(n_heads):
        p = []
        for hh in range(n_heads):
            base = hh * HD
            p.extend(range(base, base + HD, 2))
            p.extend(range(base + 1, base + HD, 2))
        return np.asarray(p)

    wq_p = np.ascontiguousarray(np.asarray(wq)[:, ab_perm(H)]).astype(MM_NP)
    wk_p = np.ascontiguousarray(np.asarray(wk)[:, ab_perm(KH)]).astype(MM_NP)
    wv_p = np.asarray(wv).astype(MM_NP)
    wo_p = np.asarray(wo).astype(MM_NP)

    rotT = np.zeros((128, 128), dtype=MM_NP)
    rotT[np.arange(64), np.arange(64) + 64] = 1.0
    rotT[np.arange(64) + 64, np.arange(64)] = -1.0
    ones = np.ones((128, 128), dtype=MM_NP)

    x = np.asarray(x, dtype=np.float32)
    freqs = np.asarray(freqs_cis, dtype=np.float64)
    tri = (np.arange(128)[:, None] <= np.arange(128)[None, :])  # k_i <= q_j

    in_maps = []
    for c in range(N_CORES):
        b, h = divmod(c, 2)
        blocks = BLOCKS_A if h == 0 else BLOCKS_B
        perm = np.concatenate(
            [np.arange(r * 128, (r + 1) * 128) for r in blocks])
        xT_c = np.ascontiguousarray(x[b][perm].T).astype(MM_NP)
        f = freqs[perm]                        # [512, 64] angles
        cos_h = np.cos(f).T.astype(np.float32)  # [64, 512]
        sin_h = np.sin(f).T.astype(np.float32)
        cosT_c = np.ascontiguousarray(np.concatenate([cos_h, cos_h], axis=0))
        sinT_c = np.ascontiguousarray(np.concatenate([sin_h, sin_h], axis=0))
        # diag masks: A: odd t -> tri, even -> ones; B: even t -> tri, odd -> 0
        dmask_c = np.empty((128, 8, 128), dtype=MM_NP)
        for t in range(8):
            if h == 0:
                dmask_c[:, t, :] = tri if (t % 2 == 1) else 1.0
            else:
                dmask_c[:, t, :] = tri if (t % 2 == 0) else 0.0
        mh0_c = np.full((128, HALF), 1.0 - h, dtype=MM_NP)
        mh1_c = np.full((128, HALF), float(h), dtype=MM_NP)
        in_maps.append({
            "xT": xT_c, "wq": wq_p, "wk": wk_p, "wv": wv_p, "wo": wo_p,
            "cosT": cosT_c, "sinT": sinT_c, "dmask": dmask_c,
            "mh0": mh0_c, "mh1": mh1_c,
            "rotT": rotT, "ones": ones,
        })
    return in_maps


def kernel(x, freqs_cis, mask, wq, wk, wv, wo):
    nc = _get_compiled()
    in_maps = _host_prep(x, freqs_cis, mask, wq, wk, wv, wo)
    res = run_bass_kernel_spmd(nc, in_maps, list(range(N_CORES)))
    out = np.empty((B, S, D), dtype=np.float32)
    for c in range(N_CORES):
        b, h = divmod(c, 2)
        blocks = BLOCKS_A if h == 0 else BLOCKS_B
        perm = np.concatenate(
            [np.arange(r * 128, (r + 1) * 128) for r in blocks])
        out[b, perm, :] = res.results[c]["out0"] + res.results[c]["out1"]
    return out


# revision 8
# speedup vs baseline: 1.5576x; 1.0752x over previous
"""GQA causal attention block (B=4, S=1024, D=4096, H=32, KH=8, HD=128) on 8
Trainium2 NeuronCores.

Sharding v3: data-parallel over (batch, q-row-parity). Each batch b is split
between cores 2b (q-row blocks [7,5,3,1], "A") and 2b+1 (blocks [6,4,2,0],
"B"), 512 query tokens per core. Each core projects Q/K/V only for its OWN
512 tokens; the pair exchanges K and V via zero-padded pairwise AllReduces
(AllGather is broken in this runtime), so K/V projection work is not
duplicated. V is projected directly in transposed [tok, vdim] layout (x tile
as the stationary matmul operand), so no on-chip transposes are needed.

Causal load balance: q-row blocks are assigned so both cores of a pair need
the same per-slot key-prefix profile C=(8,6,4,2) (in 128-row k-tiles). The
attention loop iterates original k-tile index t=7..0 with a growing q-col
prefix (128,128,256,256,384,384,512,512 cols). The key tiles live in the
gathered buffer at fixed positions (traversal table); causality inside the
last 128-col block of each prefix is applied by multiplying the exp'd probs
with a per-core 0/1 mask tile (data), everything else uses a uniform -8 exp
bias. This computes 20/32 of the dense score tiles per head.

The output projection runs as two passes of 16 heads (the second overlaps
the attention tail); the two partial outputs are summed on the host.

RoPE trick (as baseline): wq/wk columns host-permuted per head to [even|odd]
halves; rot(q) = q*cos + (S@q)*sin with a constant 128x128 swap matrix.
cos/sin tables are precomputed on host from freqs_cis.

Matmuls in fp16; softmax statistics in fp32; exp biased by -8 (cancelled by
the normalization).
"""

import numpy as np

import concourse.bass as bass
import concourse.tile as tile
from concourse import bacc, mybir
from concourse.bass_utils import run_bass_kernel_spmd

B, S, D = 4, 1024, 4096
H, KH, HD = 32, 8, 128
HALF = S // 2                    # tokens per core
N_CORES = 8
SCALE = 1.0 / float(np.sqrt(HD))
EXP_BIAS = -8.0

MM_DT = mybir.dt.float16
MM_NP = np.float16
F32 = mybir.dt.float32

DT = D // 128                    # 32 d-tiles
GROUPS = [[0, 1], [2, 3], [4, 5], [6, 7]]

# q-row blocks per core type (slot order, need-descending)
BLOCKS_A = [7, 5, 3, 1]
BLOCKS_B = [6, 4, 2, 0]
# gathered position of original k-row t: A blocks at 0..3, B at 4..7
TRAV = [None] * 8
for _s, _r in enumerate(BLOCKS_A):
    TRAV[_r] = _s
for _s, _r in enumerate(BLOCKS_B):
    TRAV[_r] = 4 + _s
# q-col prefix width when processing original k-tile t
NCOLS = [512, 512, 384, 384, 256, 256, 128, 128]

_compiled = None


def _build():
    nc = bacc.Bacc("TRN2", target_bir_lowering=False, debug=False,
                   num_devices=N_CORES)

    xT = nc.dram_tensor("xT", [D, HALF], MM_DT, kind="ExternalInput").ap()
    wq = nc.dram_tensor("wq", [D, H * HD], MM_DT, kind="ExternalInput").ap()
    wk = nc.dram_tensor("wk", [D, KH * HD], MM_DT, kind="ExternalInput").ap()
    wv = nc.dram_tensor("wv", [D, KH * HD], MM_DT, kind="ExternalInput").ap()
    wo = nc.dram_tensor("wo", [H * HD, D], MM_DT, kind="ExternalInput").ap()
    cosT_d = nc.dram_tensor("cosT", [128, HALF], F32, kind="ExternalInput").ap()
    sinT_d = nc.dram_tensor("sinT", [128, HALF], F32, kind="ExternalInput").ap()
    dmask_d = nc.dram_tensor("dmask", [128, 8, 128], MM_DT, kind="ExternalInput").ap()
    mh0_d = nc.dram_tensor("mh0", [128, HALF], MM_DT, kind="ExternalInput").ap()
    mh1_d = nc.dram_tensor("mh1", [128, HALF], MM_DT, kind="ExternalInput").ap()
    rotT_d = nc.dram_tensor("rotT", [128, 128], MM_DT, kind="ExternalInput").ap()
    ones_d = nc.dram_tensor("ones", [128, 128], MM_DT, kind="ExternalInput").ap()
    out0 = nc.dram_tensor("out0", [HALF, D], F32, kind="ExternalOutput").ap()
    out1 = nc.dram_tensor("out1", [HALF, D], F32, kind="ExternalOutput").ap()

    # pair-exchange staging (K: [hd, tok] j-tiles; V: [tok, vd] tiles)
    stageK = nc.dram_tensor("stageK", [2, 8, 128, HALF], MM_DT).ap()
    gathK = nc.dram_tensor("gathK", [2, 8, 128, HALF], MM_DT).ap()
    stageV = nc.dram_tensor("stageV", [2, 8, 128, HALF], MM_DT).ap()
    gathV = nc.dram_tensor("gathV", [2, 8, 128, HALF], MM_DT).ap()

    from contextlib import ExitStack

    es = ExitStack()
    with tile.TileContext(nc) as tc, es:
        const = es.enter_context(tc.tile_pool(name="const", bufs=1))
        xbuf = es.enter_context(tc.tile_pool(name="xbuf", bufs=1))
        qbuf = es.enter_context(tc.tile_pool(name="qbuf", bufs=1))
        abuf = es.enter_context(tc.tile_pool(name="abuf", bufs=1))
        kbuf = es.enter_context(tc.tile_pool(name="kbuf", bufs=1))
        vbuf = es.enter_context(tc.tile_pool(name="vbuf", bufs=1))
        wbuf = es.enter_context(tc.tile_pool(name="wbuf", bufs=5))
        pw = es.enter_context(tc.tile_pool(name="pw", bufs=3))
        pr = es.enter_context(tc.tile_pool(name="pr", bufs=3))
        rb = es.enter_context(tc.tile_pool(name="rb", bufs=2))
        ow = es.enter_context(tc.tile_pool(name="ow", bufs=3))
        ps_proj = es.enter_context(tc.tile_pool(name="ps_proj", bufs=4, space="PSUM"))
        ps_attn = es.enter_context(tc.tile_pool(name="ps_attn", bufs=2, space="PSUM"))
        ps_sc = es.enter_context(tc.tile_pool(name="ps_sc", bufs=2, space="PSUM"))

        # ---- x first (scalar queue; weights go on sync in parallel) ----
        xT_s = xbuf.tile([128, DT, HALF], MM_DT, tag="x")

        def load_x(dc):
            nc.scalar.dma_start(
                out=xT_s[:, dc * 4:(dc + 1) * 4, :],
                in_=xT[dc * 512:(dc + 1) * 512, :].rearrange(
                    "(t p) c -> p t c", p=128))

        load_x(0)
        load_x(1)

        # ---- constants (scalar queue) ----
        rotT = const.tile([128, 128], MM_DT, tag="rot")
        nc.scalar.dma_start(out=rotT, in_=rotT_d)
        ones = const.tile([128, 128], MM_DT, tag="ones")
        nc.scalar.dma_start(out=ones, in_=ones_d)
        cosT = const.tile([128, HALF], F32, tag="cos")
        nc.scalar.dma_start(out=cosT, in_=cosT_d)
        sinT = const.tile([128, HALF], F32, tag="sin")
        nc.scalar.dma_start(out=sinT, in_=sinT_d)
        dmask = const.tile([128, 8, 128], MM_DT, tag="dmask")
        nc.scalar.dma_start(out=dmask, in_=dmask_d)
        mh0 = const.tile([128, HALF], MM_DT, tag="mh0")
        nc.scalar.dma_start(out=mh0, in_=mh0_d)
        mh1 = const.tile([128, HALF], MM_DT, tag="mh1")
        nc.scalar.dma_start(out=mh1, in_=mh1_d)
        m8 = const.tile([128, 1], F32, tag="m8")
        nc.vector.memset(m8, EXP_BIAS)
        for dc in range(2, 8):
            load_x(dc)

        def proj_group4(w_ap, jg):
            """Project 4 j-tiles (cols jg*512..+512 of w_ap) over all of D."""
            accs = [ps_proj.tile([128, HALF], F32, tag="acc", name=f"acc{i}")
                    for i in range(4)]
            for dp in range(8):
                w_t = wbuf.tile([128, 4, HALF], MM_DT, tag="w")
                nc.sync.dma_start(
                    out=w_t,
                    in_=w_ap[dp * 512:(dp + 1) * 512,
                             jg * 512:(jg + 1) * 512].rearrange(
                                 "(t p) j -> p t j", p=128))
                for t in range(4):
                    for jj in range(4):
                        nc.tensor.matmul(
                            accs[jj], w_t[:, t, jj * 128:(jj + 1) * 128],
                            xT_s[:, dp * 4 + t, :],
                            start=(dp == 0 and t == 0),
                            stop=(dp == 7 and t == 3))
            return accs

        def projT_group4(w_ap, vg):
            """Project 4 token-tiles of (x @ w[:, vg*512:+512]) TRANSPOSED:
            accs[t4] = [128 tok, 512 vdim], x tile as stationary operand."""
            accs = [ps_proj.tile([128, HALF], F32, tag="acc", name=f"vacc{i}")
                    for i in range(4)]
            for dp in range(8):
                w_t = wbuf.tile([128, 4, HALF], MM_DT, tag="w")
                nc.sync.dma_start(
                    out=w_t,
                    in_=w_ap[dp * 512:(dp + 1) * 512,
                             vg * 512:(vg + 1) * 512].rearrange(
                                 "(t p) j -> p t j", p=128))
                for t in range(4):
                    for t4 in range(4):
                        nc.tensor.matmul(
                            accs[t4],
                            xT_s[:, dp * 4 + t, t4 * 128:(t4 + 1) * 128],
                            w_t[:, t, :],
                            start=(dp == 0 and t == 0),
                            stop=(dp == 7 and t == 3))
            return accs

        def rope(acc):
            q_s = pw.tile([128, HALF], MM_DT, tag="qs")
            nc.scalar.copy(q_s, acc)
            ps2 = ps_proj.tile([128, HALF], F32, tag="acc", name="rot")
            nc.tensor.matmul(ps2, rotT, q_s, start=True, stop=True)
            t1 = pw.tile([128, HALF], F32, tag="t1")
            nc.gpsimd.tensor_mul(t1, q_s, cosT)
            t2 = pw.tile([128, HALF], F32, tag="t2")
            nc.vector.tensor_mul(t2, ps2, sinT)
            return t1, t2

        def stage_pair(src, dst0, dst1):
            s0 = pw.tile([128, HALF], MM_DT, tag="ks0")
            nc.gpsimd.tensor_mul(s0, src, mh0)
            s1 = pw.tile([128, HALF], MM_DT, tag="ks1")
            nc.gpsimd.tensor_mul(s1, src, mh1)
            nc.scalar.dma_start(out=dst0, in_=s0)
            nc.scalar.dma_start(out=dst1, in_=s1)

        # ---- K projection (own tokens) + rope + staging + exchange ----
        for ug in range(2):
            accs = proj_group4(wk, ug)
            for jj in range(4):
                j = ug * 4 + jj
                t1, t2 = rope(accs[jj])
                rot = pw.tile([128, HALF], MM_DT, tag="rot")
                nc.gpsimd.tensor_add(rot, t1, t2)
                stage_pair(rot, stageK[0, j], stageK[1, j])
        nc.gpsimd.collective_compute(
            "AllReduce", mybir.AluOpType.add, GROUPS,
            ins=[stageK], outs=[gathK])
        kT_s = kbuf.tile([128, KH, S], MM_DT, tag="k")
        for kvh in range(KH):
            nc.sync.dma_start(out=kT_s[:, kvh, 0:512], in_=gathK[0, kvh])
            nc.sync.dma_start(out=kT_s[:, kvh, 512:1024], in_=gathK[1, kvh])

        # ---- V projection (transposed layout) + staging + exchange ----
        for vg in range(2):
            accs = projT_group4(wv, vg)
            for t4 in range(4):
                v_s = pw.tile([128, HALF], MM_DT, tag="qs")
                nc.scalar.copy(v_s, accs[t4])
                idx = t4 * 2 + vg
                stage_pair(v_s, stageV[0, idx], stageV[1, idx])
        nc.gpsimd.collective_compute(
            "AllReduce", mybir.AluOpType.add, GROUPS,
            ins=[stageV], outs=[gathV])
        v_g = vbuf.tile([128, 2, 8, HALF], MM_DT, tag="v")
        for half in range(2):
            for idx in range(8):
                nc.sync.dma_start(out=v_g[:, half, idx, :],
                                  in_=gathV[half, idx])

        def v_ap(kvh, p):
            # [128 tok, 128 vd] tile of kv-head kvh at gathered position p
            return v_g[:, p // 4, (p % 4) * 2 + (kvh // 4),
                       (kvh % 4) * 128:(kvh % 4 + 1) * 128]

        # ---- Q projection + attention, interleaved per kv-group ----
        qT = qbuf.tile([128, H, HALF], MM_DT, tag="q")
        attnT = abuf.tile([128, H, HALF], MM_DT, tag="a")

        def attend(kvh, h):
            oT = ps_attn.tile([128, HALF], F32, tag="a2", name="oT")
            sum_ps = ps_attn.tile([128, HALF], F32, tag="a2", name="sum")
            for ti, t in enumerate(range(7, -1, -1)):
                cols = NCOLS[t]
                pos = TRAV[t]
                sc = ps_sc.tile([128, HALF], F32, tag="sc")
                nc.tensor.matmul(
                    sc[:, 0:cols], kT_s[:, kvh, pos * 128:(pos + 1) * 128],
                    qT[:, h, 0:cols], start=True, stop=True)
                p_t = pr.tile([128, HALF], MM_DT, tag="pr")
                nc.scalar.activation(p_t[:, 0:cols], sc[:, 0:cols],
                                     mybir.ActivationFunctionType.Exp,
                                     bias=m8, scale=SCALE)
                nc.vector.tensor_mul(p_t[:, cols - 128:cols],
                                     p_t[:, cols - 128:cols], dmask[:, t, :])
                nc.tensor.matmul(oT[:, 0:cols], v_ap(kvh, pos),
                                 p_t[:, 0:cols], start=(ti == 0),
                                 stop=(ti == 7))
                nc.tensor.matmul(sum_ps[:, 0:cols], ones, p_t[:, 0:cols],
                                 start=(ti == 0), stop=(ti == 7))
            rB2 = rb.tile([128, HALF], F32, tag="rb2")
            nc.vector.reciprocal_approx_fast(rB2, sum_ps)
            nc.vector.tensor_mul(attnT[:, h, :], oT, rB2)

        for g in range(8):
            accs = proj_group4(wq, g)
            for jj in range(4):
                t1, t2 = rope(accs[jj])
                nc.gpsimd.tensor_add(qT[:, g * 4 + jj, :], t1, t2)
            for hh in range(4):
                attend(g, g * 4 + hh)

        # ---- output projection: two passes of 16 heads ----
        def out_pass(p, dst):
            for djg in range(8):
                accs = [ps_proj.tile([128, HALF], F32, tag="acc",
                                     name=f"oacc{i}") for i in range(4)]
                for hq in range(4):
                    wo_t = wbuf.tile([128, 4, HALF], MM_DT, tag="w")
                    nc.sync.dma_start(
                        out=wo_t,
                        in_=wo[(p * 4 + hq) * 512:(p * 4 + hq + 1) * 512,
                               djg * 512:(djg + 1) * 512].rearrange(
                                   "(t p) j -> p t j", p=128))
                    for t in range(4):
                        hd = p * 16 + hq * 4 + t
                        for t4 in range(4):
                            nc.tensor.matmul(
                                accs[t4],
                                attnT[:, hd, t4 * 128:(t4 + 1) * 128],
                                wo_t[:, t, :],
                                start=(hq == 0 and t == 0),
                                stop=(hq == 3 and t == 3))
                for t4 in range(4):
                    o_s = ow.tile([128, HALF], F32, tag="ow")
                    nc.scalar.copy(o_s, accs[t4])
                    nc.scalar.dma_start(
                        out=dst[t4 * 128:(t4 + 1) * 128,
                                djg * 512:(djg + 1) * 512],
                        in_=o_s)

        out_pass(0, out0)
        out_pass(1, out1)

    nc.compile()
    return nc


def _get_compiled():
    global _compiled
    if _compiled is None:
        _compiled = _build()
    return _compiled


def _host_prep(x, freqs_cis, mask, wq, wk, wv, wo):
    """Shard + lay out inputs per core. Core 2b+h: batch b, q-row parity h."""
    del mask  # causal structure is hardcoded in the per-core mask tiles

    def ab_perm(n_heads):
        p = []
        for hh in range(n_heads):
            base = hh * HD
            p.extend(range(base, base + HD, 2))
            p.extend(range(base + 1, base + HD, 2))
        return np.asarray(p)

    wq_p = np.ascontiguousarray(np.asarray(wq)[:, ab_perm(H)]).astype(MM_NP)
    wk_p = np.ascontiguousarray(np.asarray(wk)[:, ab_perm(KH)]).astype(MM_NP)
    wv_p = np.asarray(wv).astype(MM_NP)
    wo_p = np.asarray(wo).astype(MM_NP)

    rotT = np.zeros((128, 128), dtype=MM_NP)
    rotT[np.arange(64), np.arange(64) + 64] = 1.0
    rotT[np.arange(64) + 64, np.arange(64)] = -1.0
    ones = np.ones((128, 128), dtype=MM_NP)

    x = np.asarray(x, dtype=np.float32)
    freqs = np.asarray(freqs_cis, dtype=np.float64)
    tri = (np.arange(128)[:, None] <= np.arange(128)[None, :])  # k_i <= q_j

    in_maps = []
    for c in range(N_CORES):
        b, h = divmod(c, 2)
        blocks = BLOCKS_A if h == 0 else BLOCKS_B
        perm = np.concatenate(
            [np.arange(r * 128, (r + 1) * 128) for r in blocks])
        xT_c = np.ascontiguousarray(x[b][perm].T).astype(MM_NP)
        f = freqs[perm]                        # [512, 64] angles
        cos_h = np.cos(f).T.astype(np.float32)  # [64, 512]
        sin_h = np.sin(f).T.astype(np.float32)
        cosT_c = np.ascontiguousarray(np.concatenate([cos_h, cos_h], axis=0))
        sinT_c = np.ascontiguousarray(np.concatenate([sin_h, sin_h], axis=0))
        # diag masks: A: odd t -> tri, even -> ones; B: even t -> tri, odd -> 0
        dmask_c = np.empty((128, 8, 128), dtype=MM_NP)
        for t in range(8):
            if h == 0:
                dmask_c[:, t, :] = tri if (t % 2 == 1) else 1.0
            else:
                dmask_c[:, t, :] = tri if (t % 2 == 0) else 0.0
        mh0_c = np.full((128, HALF), 1.0 - h, dtype=MM_NP)
        mh1_c = np.full((128, HALF), float(h), dtype=MM_NP)
        in_maps.append({
            "xT": xT_c, "wq": wq_p, "wk": wk_p, "wv": wv_p, "wo": wo_p,
            "cosT": cosT_c, "sinT": sinT_c, "dmask": dmask_c,
            "mh0": mh0_c, "mh1": mh1_c,
            "rotT": rotT, "ones": ones,
        })
    return in_maps


def kernel(x, freqs_cis, mask, wq, wk, wv, wo):
    nc = _get_compiled()
    in_maps = _host_prep(x, freqs_cis, mask, wq, wk, wv, wo)
    res = run_bass_kernel_spmd(nc, in_maps, list(range(N_CORES)))
    out = np.empty((B, S, D), dtype=np.float32)
    for c in range(N_CORES):
        b, h = divmod(c, 2)
        blocks = BLOCKS_A if h == 0 else BLOCKS_B
        perm = np.concatenate(
            [np.arange(r * 128, (r + 1) * 128) for r in blocks])
        out[b, perm, :] = res.results[c]["out0"] + res.results[c]["out1"]
    return out


# revision 11
# speedup vs baseline: 1.6572x; 1.0640x over previous
"""GQA causal attention block (B=4, S=1024, D=4096, H=32, KH=8, HD=128) on 8
Trainium2 NeuronCores.

Sharding v3: data-parallel over (batch, q-row-parity). Each batch b is split
between cores 2b (q-row blocks [7,5,3,1], "A") and 2b+1 (blocks [6,4,2,0],
"B"), 512 query tokens per core. Each core projects Q/K/V only for its OWN
512 tokens; the pair exchanges K and V via zero-padded pairwise AllReduces
(AllGather is broken in this runtime), so K/V projection work is not
duplicated. V is projected directly in transposed [tok, vdim] layout (x tile
as the stationary matmul operand), so no on-chip transposes are needed.

Causal load balance: q-row blocks are assigned so both cores of a pair need
the same per-slot key-prefix profile C=(8,6,4,2) (in 128-row k-tiles). The
attention loop iterates original k-tile index t=7..0 with a growing q-col
prefix (128,128,256,256,384,384,512,512 cols). The key tiles live in the
gathered buffer at fixed positions (traversal table); causality inside the
last 128-col block of each prefix is applied by multiplying the exp'd probs
with a per-core 0/1 mask tile (data), everything else uses a uniform -8 exp
bias. This computes 20/32 of the dense score tiles per head.

The output projection runs as two passes of 16 heads (the second overlaps
the attention tail); the two partial outputs are summed on the host.

RoPE trick (as baseline): wq/wk columns host-permuted per head to [even|odd]
halves; rot(q) = q*cos + (S@q)*sin with a constant 128x128 swap matrix.
cos/sin tables are precomputed on host from freqs_cis.

Matmuls in fp16; softmax statistics in fp32; exp biased by -8 (cancelled by
the normalization).
"""

import numpy as np

import concourse.bass as bass
import concourse.tile as tile
from concourse import bacc, mybir
from concourse.bass_utils import run_bass_kernel_spmd

B, S, D = 4, 1024, 4096
H, KH, HD = 32, 8, 128
HALF = S // 2                    # tokens per core
N_CORES = 8
SCALE = 1.0 / float(np.sqrt(HD))
EXP_BIAS = -8.0

MM_DT = mybir.dt.float16
MM_NP = np.float16
F32 = mybir.dt.float32

DT = D // 128                    # 32 d-tiles
GROUPS = [[0, 1], [2, 3], [4, 5], [6, 7]]

# q-row blocks per core type (slot order, need-descending)
BLOCKS_A = [7, 5, 3, 1]
BLOCKS_B = [6, 4, 2, 0]
# gathered position of original k-row t: A blocks at 0..3, B at 4..7
TRAV = [None] * 8
for _s, _r in enumerate(BLOCKS_A):
    TRAV[_r] = _s
for _s, _r in enumerate(BLOCKS_B):
    TRAV[_r] = 4 + _s
# q-col prefix width when processing original k-tile t
NCOLS = [512, 512, 384, 384, 256, 256, 128, 128]

_compiled = None


def _build():
    nc = bacc.Bacc("TRN2", target_bir_lowering=False, debug=False,
                   num_devices=N_CORES)

    xT = nc.dram_tensor("xT", [D, HALF], MM_DT, kind="ExternalInput").ap()
    wq = nc.dram_tensor("wq", [D, H * HD], MM_DT, kind="ExternalInput").ap()
    wk = nc.dram_tensor("wk", [D, KH * HD], MM_DT, kind="ExternalInput").ap()
    wv = nc.dram_tensor("wv", [D, KH * HD], MM_DT, kind="ExternalInput").ap()
    wo = nc.dram_tensor("wo", [H * HD, D], MM_DT, kind="ExternalInput").ap()
    cosT_d = nc.dram_tensor("cosT", [128, HALF], F32, kind="ExternalInput").ap()
    sinT_d = nc.dram_tensor("sinT", [128, HALF], F32, kind="ExternalInput").ap()
    dmask_d = nc.dram_tensor("dmask", [128, 8, 128], MM_DT, kind="ExternalInput").ap()
    mh0_d = nc.dram_tensor("mh0", [128, HALF], MM_DT, kind="ExternalInput").ap()
    mh1_d = nc.dram_tensor("mh1", [128, HALF], MM_DT, kind="ExternalInput").ap()
    rotT_d = nc.dram_tensor("rotT", [128, 128], MM_DT, kind="ExternalInput").ap()
    ones_d = nc.dram_tensor("ones", [128, 128], MM_DT, kind="ExternalInput").ap()
    out0 = nc.dram_tensor("out0", [HALF, D], F32, kind="ExternalOutput").ap()
    out1 = nc.dram_tensor("out1", [HALF, D], F32, kind="ExternalOutput").ap()

    # pair-exchange staging (K: [hd, tok] j-tiles; V: [tok, vd] tiles)
    stageK = nc.dram_tensor("stageK", [2, 8, 128, HALF], MM_DT).ap()
    gathK = nc.dram_tensor("gathK", [2, 8, 128, HALF], MM_DT).ap()
    stageV = nc.dram_tensor("stageV", [2, 8, 128, HALF], MM_DT).ap()
    gathV = nc.dram_tensor("gathV", [2, 8, 128, HALF], MM_DT).ap()

    from contextlib import ExitStack

    es = ExitStack()
    with tile.TileContext(nc) as tc, es:
        const = es.enter_context(tc.tile_pool(name="const", bufs=1))
        xbuf = es.enter_context(tc.tile_pool(name="xbuf", bufs=1))
        qbuf = es.enter_context(tc.tile_pool(name="qbuf", bufs=1))
        abuf = es.enter_context(tc.tile_pool(name="abuf", bufs=1))
        kbuf = es.enter_context(tc.tile_pool(name="kbuf", bufs=1))
        vbuf = es.enter_context(tc.tile_pool(name="vbuf", bufs=1))
        wbuf = es.enter_context(tc.tile_pool(name="wbuf", bufs=5))
        pw = es.enter_context(tc.tile_pool(name="pw", bufs=3))
        pr = es.enter_context(tc.tile_pool(name="pr", bufs=3))
        rb = es.enter_context(tc.tile_pool(name="rb", bufs=2))
        ow = es.enter_context(tc.tile_pool(name="ow", bufs=3))
        ps_proj = es.enter_context(tc.tile_pool(name="ps_proj", bufs=4, space="PSUM"))
        ps_attn = es.enter_context(tc.tile_pool(name="ps_attn", bufs=2, space="PSUM"))
        ps_sc = es.enter_context(tc.tile_pool(name="ps_sc", bufs=2, space="PSUM"))

        # ---- x first (scalar queue; weights go on sync in parallel) ----
        xT_s = xbuf.tile([128, DT, HALF], MM_DT, tag="x")

        def load_x(dc):
            nc.scalar.dma_start(
                out=xT_s[:, dc * 4:(dc + 1) * 4, :],
                in_=xT[dc * 512:(dc + 1) * 512, :].rearrange(
                    "(t p) c -> p t c", p=128))

        load_x(0)
        load_x(1)

        # ---- constants (scalar queue) ----
        rotT = const.tile([128, 128], MM_DT, tag="rot")
        nc.scalar.dma_start(out=rotT, in_=rotT_d)
        ones = const.tile([128, 128], MM_DT, tag="ones")
        nc.scalar.dma_start(out=ones, in_=ones_d)
        cosT = const.tile([128, HALF], F32, tag="cos")
        nc.scalar.dma_start(out=cosT, in_=cosT_d)
        sinT = const.tile([128, HALF], F32, tag="sin")
        nc.scalar.dma_start(out=sinT, in_=sinT_d)
        dmask = const.tile([128, 8, 128], MM_DT, tag="dmask")
        nc.scalar.dma_start(out=dmask, in_=dmask_d)
        mh0 = const.tile([128, HALF], MM_DT, tag="mh0")
        nc.scalar.dma_start(out=mh0, in_=mh0_d)
        mh1 = const.tile([128, HALF], MM_DT, tag="mh1")
        nc.scalar.dma_start(out=mh1, in_=mh1_d)
        m8 = const.tile([128, 1], F32, tag="m8")
        nc.vector.memset(m8, EXP_BIAS)
        for dc in range(2, 8):
            load_x(dc)

        def proj_group4(w_ap, jg):
            """Project 4 j-tiles (cols jg*512..+512 of w_ap) over all of D."""
            accs = [ps_proj.tile([128, HALF], F32, tag="acc", name=f"acc{i}")
                    for i in range(4)]
            for dp in range(8):
                w_t = wbuf.tile([128, 4, HALF], MM_DT, tag="w")
                nc.sync.dma_start(
                    out=w_t,
                    in_=w_ap[dp * 512:(dp + 1) * 512,
                             jg * 512:(jg + 1) * 512].rearrange(
                                 "(t p) j -> p t j", p=128))
                for t in range(4):
                    for jj in range(4):
                        nc.tensor.matmul(
                            accs[jj], w_t[:, t, jj * 128:(jj + 1) * 128],
                            xT_s[:, dp * 4 + t, :],
                            start=(dp == 0 and t == 0),
                            stop=(dp == 7 and t == 3))
            return accs

        def projT_group4(w_ap, vg):
            """Project 4 token-tiles of (x @ w[:, vg*512:+512]) TRANSPOSED:
            accs[t4] = [128 tok, 512 vdim], x tile as stationary operand."""
            accs = [ps_proj.tile([128, HALF], F32, tag="acc", name=f"vacc{i}")
                    for i in range(4)]
            for dp in range(8):
                w_t = wbuf.tile([128, 4, HALF], MM_DT, tag="w")
                nc.sync.dma_start(
                    out=w_t,
                    in_=w_ap[dp * 512:(dp + 1) * 512,
                             vg * 512:(vg + 1) * 512].rearrange(
                                 "(t p) j -> p t j", p=128))
                for t in range(4):
                    for t4 in range(4):
                        nc.tensor.matmul(
                            accs[t4],
                            xT_s[:, dp * 4 + t, t4 * 128:(t4 + 1) * 128],
                            w_t[:, t, :],
                            start=(dp == 0 and t == 0),
                            stop=(dp == 7 and t == 3))
            return accs

        def rope(acc):
            q_s = pw.tile([128, HALF], MM_DT, tag="qs")
            nc.scalar.copy(q_s, acc)
            ps2 = ps_proj.tile([128, HALF], F32, tag="acc", name="rot")
            nc.tensor.matmul(ps2, rotT, q_s, start=True, stop=True)
            t1 = pw.tile([128, HALF], F32, tag="t1")
            nc.gpsimd.tensor_mul(t1, q_s, cosT)
            t2 = pw.tile([128, HALF], F32, tag="t2")
            nc.vector.tensor_mul(t2, ps2, sinT)
            return t1, t2

        def stage_pair(src, dst0, dst1):
            s0 = pw.tile([128, HALF], MM_DT, tag="ks0")
            nc.gpsimd.tensor_mul(s0, src, mh0)
            s1 = pw.tile([128, HALF], MM_DT, tag="ks1")
            nc.gpsimd.tensor_mul(s1, src, mh1)
            nc.scalar.dma_start(out=dst0, in_=s0)
            nc.scalar.dma_start(out=dst1, in_=s1)

        # ---- K projection (own tokens) + rope + staging + exchange ----
        for ug in range(2):
            accs = proj_group4(wk, ug)
            for jj in range(4):
                j = ug * 4 + jj
                t1, t2 = rope(accs[jj])
                rot = pw.tile([128, HALF], MM_DT, tag="rot")
                nc.gpsimd.tensor_add(rot, t1, t2)
                stage_pair(rot, stageK[0, j], stageK[1, j])
        nc.gpsimd.collective_compute(
            "AllReduce", mybir.AluOpType.add, GROUPS,
            ins=[stageK], outs=[gathK])
        kT_s = kbuf.tile([128, KH, S], MM_DT, tag="k")

        # ---- V projection (transposed layout) + staging + exchange ----
        for vg in range(2):
            accs = projT_group4(wv, vg)
            for t4 in range(4):
                v_s = pw.tile([128, HALF], MM_DT, tag="qs")
                nc.scalar.copy(v_s, accs[t4])
                idx = t4 * 2 + vg
                stage_pair(v_s, stageV[0, idx], stageV[1, idx])
        nc.gpsimd.collective_compute(
            "AllReduce", mybir.AluOpType.add, GROUPS,
            ins=[stageV], outs=[gathV])
        v_g = vbuf.tile([128, 2, 8, HALF], MM_DT, tag="v")

        def v_ap(kvh, p):
            # [128 tok, 128 vd] tile of kv-head kvh at gathered position p
            return v_g[:, p // 4, (p % 4) * 2 + (kvh // 4),
                       (kvh % 4) * 128:(kvh % 4 + 1) * 128]

        # ---- Q projection + attention, interleaved per kv-group ----
        qT = qbuf.tile([128, H, HALF], MM_DT, tag="q")
        attnT = abuf.tile([128, H, HALF], MM_DT, tag="a")

        def attend(kvh, h):
            oT = ps_attn.tile([128, HALF], F32, tag="a2", name="oT")
            sum_ps = ps_attn.tile([128, HALF], F32, tag="a2", name="sum")
            for ti, t in enumerate(range(7, -1, -1)):
                cols = NCOLS[t]
                pos = TRAV[t]
                sc = ps_sc.tile([128, HALF], F32, tag="sc")
                nc.tensor.matmul(
                    sc[:, 0:cols], kT_s[:, kvh, pos * 128:(pos + 1) * 128],
                    qT[:, h, 0:cols], start=True, stop=True)
                p_t = pr.tile([128, HALF], MM_DT, tag="pr")
                nc.scalar.activation(p_t[:, 0:cols], sc[:, 0:cols],
                                     mybir.ActivationFunctionType.Exp,
                                     bias=m8, scale=SCALE)
                nc.vector.tensor_mul(p_t[:, cols - 128:cols],
                                     p_t[:, cols - 128:cols], dmask[:, t, :])
                nc.tensor.matmul(oT[:, 0:cols], v_ap(kvh, pos),
                                 p_t[:, 0:cols], start=(ti == 0),
                                 stop=(ti == 7))
                nc.tensor.matmul(sum_ps[:, 0:cols], ones, p_t[:, 0:cols],
                                 start=(ti == 0), stop=(ti == 7))
            rB2 = rb.tile([128, HALF], F32, tag="rb2")
            nc.vector.reciprocal_approx_fast(rB2, sum_ps)
            nc.vector.tensor_mul(attnT[:, h, :], oT, rB2)

        for g in range(8):
            accs = proj_group4(wq, g)
            for jj in range(4):
                t1, t2 = rope(accs[jj])
                nc.gpsimd.tensor_add(qT[:, g * 4 + jj, :], t1, t2)
            if g == 3:
                # CC_K long done by the time the scalar queue reaches these
                for kvh in range(KH):
                    nc.scalar.dma_start(out=kT_s[:, kvh, 0:512],
                                        in_=gathK[0, kvh])
                    nc.scalar.dma_start(out=kT_s[:, kvh, 512:1024],
                                        in_=gathK[1, kvh])
            if g == 4:
                for half in range(2):
                    for idx in range(8):
                        nc.scalar.dma_start(out=v_g[:, half, idx, :],
                                            in_=gathV[half, idx])
            if g >= 5:
                for h in range(2 * (g - 5), 2 * (g - 5) + 2):
                    attend(h // 4, h)

        for h in range(6, 16):
            attend(h // 4, h)

        def out_djg(p, dst, djg):
            accs = [ps_proj.tile([128, HALF], F32, tag="acc",
                                 name=f"oacc{i}") for i in range(4)]
            for hq in range(4):
                wo_t = wbuf.tile([128, 4, HALF], MM_DT, tag="w")
                nc.sync.dma_start(
                    out=wo_t,
                    in_=wo[(p * 4 + hq) * 512:(p * 4 + hq + 1) * 512,
                           djg * 512:(djg + 1) * 512].rearrange(
                               "(t p) j -> p t j", p=128))
                for t in range(4):
                    hd = p * 16 + hq * 4 + t
                    for t4 in range(4):
                        nc.tensor.matmul(
                            accs[t4],
                            attnT[:, hd, t4 * 128:(t4 + 1) * 128],
                            wo_t[:, t, :],
                            start=(hq == 0 and t == 0),
                            stop=(hq == 3 and t == 3))
            for t4 in range(4):
                o_s = ow.tile([128, HALF], F32, tag="ow")
                nc.scalar.copy(o_s, accs[t4])
                nc.scalar.dma_start(
                    out=dst[t4 * 128:(t4 + 1) * 128,
                            djg * 512:(djg + 1) * 512],
                    in_=o_s)

        for djg in range(8):
            for h in (16 + 2 * djg, 17 + 2 * djg):
                attend(h // 4, h)
            out_djg(0, out0, djg)
        for djg in range(8):
            out_djg(1, out1, djg)

    nc.compile()
    return nc


def _get_compiled():
    global _compiled
    if _compiled is None:
        _compiled = _build()
    return _compiled


def _host_prep(x, freqs_cis, mask, wq, wk, wv, wo):
    """Shard + lay out inputs per core. Core 2b+h: batch b, q-row parity h."""
    del mask  # causal structure is hardcoded in the per-core mask tiles

    def ab_perm(n_heads):
        p = []
        for hh in range(n_heads):
            base = hh * HD
            p.extend(range(base, base + HD, 2))
            p.extend(range(base + 1, base + HD, 2))
        return np.asarray(p)

    wq_p = np.ascontiguousarray(np.asarray(wq)[:, ab_perm(H)]).astype(MM_NP)
    wk_p = np.ascontiguousarray(np.asarray(wk)[:, ab_perm(KH)]).astype(MM_NP)
    wv_p = np.asarray(wv).astype(MM_NP)
    wo_p = np.asarray(wo).astype(MM_NP)

    rotT = np.zeros((128, 128), dtype=MM_NP)
    rotT[np.arange(64), np.arange(64) + 64] = 1.0
    rotT[np.arange(64) + 64, np.arange(64)] = -1.0
    ones = np.ones((128, 128), dtype=MM_NP)

    x = np.asarray(x, dtype=np.float32)
    freqs = np.asarray(freqs_cis, dtype=np.float64)
    tri = (np.arange(128)[:, None] <= np.arange(128)[None, :])  # k_i <= q_j

    in_maps = []
    for c in range(N_CORES):
        b, h = divmod(c, 2)
        blocks = BLOCKS_A if h == 0 else BLOCKS_B
        perm = np.concatenate(
            [np.arange(r * 128, (r + 1) * 128) for r in blocks])
        xT_c = np.ascontiguousarray(x[b][perm].T).astype(MM_NP)
        f = freqs[perm]                        # [512, 64] angles
        cos_h = np.cos(f).T.astype(np.float32)  # [64, 512]
        sin_h = np.sin(f).T.astype(np.float32)
        cosT_c = np.ascontiguousarray(np.concatenate([cos_h, cos_h], axis=0))
        sinT_c = np.ascontiguousarray(np.concatenate([sin_h, sin_h], axis=0))
        # diag masks: A: odd t -> tri, even -> ones; B: even t -> tri, odd -> 0
        dmask_c = np.empty((128, 8, 128), dtype=MM_NP)
        for t in range(8):
            if h == 0:
                dmask_c[:, t, :] = tri if (t % 2 == 1) else 1.0
            else:
                dmask_c[:, t, :] = tri if (t % 2 == 0) else 0.0
        mh0_c = np.full((128, HALF), 1.0 - h, dtype=MM_NP)
        mh1_c = np.full((128, HALF), float(h), dtype=MM_NP)
        in_maps.append({
            "xT": xT_c, "wq": wq_p, "wk": wk_p, "wv": wv_p, "wo": wo_p,
            "cosT": cosT_c, "sinT": sinT_c, "dmask": dmask_c,
            "mh0": mh0_c, "mh1": mh1_c,
            "rotT": rotT, "ones": ones,
        })
    return in_maps


def kernel(x, freqs_cis, mask, wq, wk, wv, wo):
    nc = _get_compiled()
    in_maps = _host_prep(x, freqs_cis, mask, wq, wk, wv, wo)
    res = run_bass_kernel_spmd(nc, in_maps, list(range(N_CORES)))
    out = np.empty((B, S, D), dtype=np.float32)
    for c in range(N_CORES):
        b, h = divmod(c, 2)
        blocks = BLOCKS_A if h == 0 else BLOCKS_B
        perm = np.concatenate(
            [np.arange(r * 128, (r + 1) * 128) for r in blocks])
        out[b, perm, :] = res.results[c]["out0"] + res.results[c]["out1"]
    return out


# revision 12
# speedup vs baseline: 1.7511x; 1.0566x over previous
"""GQA causal attention block (B=4, S=1024, D=4096, H=32, KH=8, HD=128) on 8
Trainium2 NeuronCores.

Sharding v3: data-parallel over (batch, q-row-parity). Each batch b is split
between cores 2b (q-row blocks [7,5,3,1], "A") and 2b+1 (blocks [6,4,2,0],
"B"), 512 query tokens per core. Each core projects Q/K/V only for its OWN
512 tokens; the pair exchanges K and V via zero-padded pairwise AllReduces
(AllGather is broken in this runtime), so K/V projection work is not
duplicated. V is projected directly in transposed [tok, vdim] layout (x tile
as the stationary matmul operand), so no on-chip transposes are needed.

Causal load balance: q-row blocks are assigned so both cores of a pair need
the same per-slot key-prefix profile C=(8,6,4,2) (in 128-row k-tiles). The
attention loop iterates original k-tile index t=7..0 with a growing q-col
prefix (128,128,256,256,384,384,512,512 cols). The key tiles live in the
gathered buffer at fixed positions (traversal table); causality inside the
last 128-col block of each prefix is applied by multiplying the exp'd probs
with a per-core 0/1 mask tile (data), everything else uses a uniform -8 exp
bias. This computes 20/32 of the dense score tiles per head.

The output projection runs as two passes of 16 heads (the second overlaps
the attention tail); the two partial outputs are summed on the host.

RoPE trick (as baseline): wq/wk columns host-permuted per head to [even|odd]
halves; rot(q) = q*cos + (S@q)*sin with a constant 128x128 swap matrix.
cos/sin tables are precomputed on host from freqs_cis.

Matmuls in fp16; softmax statistics in fp32; exp biased by -8 (cancelled by
the normalization).
"""

import numpy as np

import concourse.bass as bass
import concourse.tile as tile
from concourse import bacc, mybir
from concourse.bass_utils import run_bass_kernel_spmd

B, S, D = 4, 1024, 4096
H, KH, HD = 32, 8, 128
HALF = S // 2                    # tokens per core
N_CORES = 8
SCALE = 1.0 / float(np.sqrt(HD))
EXP_BIAS = -8.0

MM_DT = mybir.dt.float16
MM_NP = np.float16
F32 = mybir.dt.float32

DT = D // 128                    # 32 d-tiles
GROUPS = [[0, 1], [2, 3], [4, 5], [6, 7]]

# q-row blocks per core type (slot order, need-descending)
BLOCKS_A = [7, 5, 3, 1]
BLOCKS_B = [6, 4, 2, 0]
# gathered position of original k-row t: A blocks at 0..3, B at 4..7
TRAV = [None] * 8
for _s, _r in enumerate(BLOCKS_A):
    TRAV[_r] = _s
for _s, _r in enumerate(BLOCKS_B):
    TRAV[_r] = 4 + _s
# q-col prefix width when processing original k-tile t
NCOLS = [512, 512, 384, 384, 256, 256, 128, 128]

_compiled = None


def _build():
    nc = bacc.Bacc("TRN2", target_bir_lowering=False, debug=False,
                   num_devices=N_CORES)

    xT = nc.dram_tensor("xT", [D, HALF], MM_DT, kind="ExternalInput").ap()
    wq = nc.dram_tensor("wq", [D, H * HD], MM_DT, kind="ExternalInput").ap()
    wk = nc.dram_tensor("wk", [D, KH * HD], MM_DT, kind="ExternalInput").ap()
    wv = nc.dram_tensor("wv", [D, KH * HD], MM_DT, kind="ExternalInput").ap()
    wo = nc.dram_tensor("wo", [H * HD, D], MM_DT, kind="ExternalInput").ap()
    cosT_d = nc.dram_tensor("cosT", [128, HALF], F32, kind="ExternalInput").ap()
    sinT_d = nc.dram_tensor("sinT", [128, HALF], F32, kind="ExternalInput").ap()
    dmask_d = nc.dram_tensor("dmask", [128, 8, 128], MM_DT, kind="ExternalInput").ap()
    mh0_d = nc.dram_tensor("mh0", [128, HALF], MM_DT, kind="ExternalInput").ap()
    mh1_d = nc.dram_tensor("mh1", [128, HALF], MM_DT, kind="ExternalInput").ap()
    rotT_d = nc.dram_tensor("rotT", [128, 128], MM_DT, kind="ExternalInput").ap()
    ones_d = nc.dram_tensor("ones", [128, 128], MM_DT, kind="ExternalInput").ap()
    out0 = nc.dram_tensor("out0", [HALF, D], F32, kind="ExternalOutput").ap()
    out1 = nc.dram_tensor("out1", [HALF, D], F32, kind="ExternalOutput").ap()

    # pair-exchange staging (K: [hd, tok] j-tiles; V: [tok, vd] tiles)
    stageK = nc.dram_tensor("stageK", [2, 8, 128, HALF], MM_DT).ap()
    gathK = nc.dram_tensor("gathK", [2, 8, 128, HALF], MM_DT).ap()
    stageV = nc.dram_tensor("stageV", [2, 8, 128, HALF], MM_DT).ap()
    gathV = nc.dram_tensor("gathV", [2, 8, 128, HALF], MM_DT).ap()

    from contextlib import ExitStack

    es = ExitStack()
    with tile.TileContext(nc) as tc, es:
        const = es.enter_context(tc.tile_pool(name="const", bufs=1))
        xbuf = es.enter_context(tc.tile_pool(name="xbuf", bufs=1))
        qbuf = es.enter_context(tc.tile_pool(name="qbuf", bufs=1))
        abuf = es.enter_context(tc.tile_pool(name="abuf", bufs=1))
        kbuf = es.enter_context(tc.tile_pool(name="kbuf", bufs=1))
        vbuf = es.enter_context(tc.tile_pool(name="vbuf", bufs=1))
        wbuf = es.enter_context(tc.tile_pool(name="wbuf", bufs=6))
        pw = es.enter_context(tc.tile_pool(name="pw", bufs=3))
        pr = es.enter_context(tc.tile_pool(name="pr", bufs=3))
        rb = es.enter_context(tc.tile_pool(name="rb", bufs=2))
        ow = es.enter_context(tc.tile_pool(name="ow", bufs=3))
        ps_proj = es.enter_context(tc.tile_pool(name="ps_proj", bufs=4, space="PSUM"))
        ps_attn = es.enter_context(tc.tile_pool(name="ps_attn", bufs=2, space="PSUM"))
        ps_sc = es.enter_context(tc.tile_pool(name="ps_sc", bufs=2, space="PSUM"))

        # ---- x first (scalar queue; weights go on sync in parallel) ----
        xT_s = xbuf.tile([128, DT, HALF], MM_DT, tag="x")

        def load_x(dc):
            nc.scalar.dma_start(
                out=xT_s[:, dc * 4:(dc + 1) * 4, :],
                in_=xT[dc * 512:(dc + 1) * 512, :].rearrange(
                    "(t p) c -> p t c", p=128))

        nc.scalar.dma_start(
            out=xT_s[:, 0:2, :],
            in_=xT[0:256, :].rearrange("(t p) c -> p t c", p=128))
        nc.scalar.dma_start(
            out=xT_s[:, 2:4, :],
            in_=xT[256:512, :].rearrange("(t p) c -> p t c", p=128))
        load_x(1)

        # ---- constants (scalar queue) ----
        rotT = const.tile([128, 128], MM_DT, tag="rot")
        nc.scalar.dma_start(out=rotT, in_=rotT_d)
        ones = const.tile([128, 128], MM_DT, tag="ones")
        nc.scalar.dma_start(out=ones, in_=ones_d)
        cosT = const.tile([128, HALF], F32, tag="cos")
        nc.scalar.dma_start(out=cosT, in_=cosT_d)
        sinT = const.tile([128, HALF], F32, tag="sin")
        nc.scalar.dma_start(out=sinT, in_=sinT_d)
        dmask = const.tile([128, 8, 128], MM_DT, tag="dmask")
        nc.scalar.dma_start(out=dmask, in_=dmask_d)
        mh0 = const.tile([128, HALF], MM_DT, tag="mh0")
        nc.scalar.dma_start(out=mh0, in_=mh0_d)
        mh1 = const.tile([128, HALF], MM_DT, tag="mh1")
        nc.scalar.dma_start(out=mh1, in_=mh1_d)
        m8 = const.tile([128, 1], F32, tag="m8")
        nc.vector.memset(m8, EXP_BIAS)
        for dc in range(2, 8):
            load_x(dc)

        def proj_group4(w_ap, jg):
            """Project 4 j-tiles (cols jg*512..+512 of w_ap) over all of D."""
            accs = [ps_proj.tile([128, HALF], F32, tag="acc", name=f"acc{i}")
                    for i in range(4)]
            for dp in range(8):
                w_t = wbuf.tile([128, 4, HALF], MM_DT, tag="w")
                nc.sync.dma_start(
                    out=w_t,
                    in_=w_ap[dp * 512:(dp + 1) * 512,
                             jg * 512:(jg + 1) * 512].rearrange(
                                 "(t p) j -> p t j", p=128))
                for t in range(4):
                    for jj in range(4):
                        nc.tensor.matmul(
                            accs[jj], w_t[:, t, jj * 128:(jj + 1) * 128],
                            xT_s[:, dp * 4 + t, :],
                            start=(dp == 0 and t == 0),
                            stop=(dp == 7 and t == 3))
            return accs

        def projT_group4(w_ap, vg):
            """Project 4 token-tiles of (x @ w[:, vg*512:+512]) TRANSPOSED:
            accs[t4] = [128 tok, 512 vdim], x tile as stationary operand."""
            accs = [ps_proj.tile([128, HALF], F32, tag="acc", name=f"vacc{i}")
                    for i in range(4)]
            for dp in range(8):
                w_t = wbuf.tile([128, 4, HALF], MM_DT, tag="w")
                nc.sync.dma_start(
                    out=w_t,
                    in_=w_ap[dp * 512:(dp + 1) * 512,
                             vg * 512:(vg + 1) * 512].rearrange(
                                 "(t p) j -> p t j", p=128))
                for t in range(4):
                    for t4 in range(4):
                        nc.tensor.matmul(
                            accs[t4],
                            xT_s[:, dp * 4 + t, t4 * 128:(t4 + 1) * 128],
                            w_t[:, t, :],
                            start=(dp == 0 and t == 0),
                            stop=(dp == 7 and t == 3))
            return accs

        def rope(acc):
            q_s = pw.tile([128, HALF], MM_DT, tag="qs")
            nc.scalar.copy(q_s, acc)
            ps2 = ps_proj.tile([128, HALF], F32, tag="acc", name="rot")
            nc.tensor.matmul(ps2, rotT, q_s, start=True, stop=True)
            t1 = pw.tile([128, HALF], F32, tag="t1")
            nc.vector.tensor_mul(t1, q_s, cosT)
            t2 = pw.tile([128, HALF], F32, tag="t2")
            nc.vector.tensor_mul(t2, ps2, sinT)
            return t1, t2

        def stage_pair(src, dst0, dst1):
            s0 = pw.tile([128, HALF], MM_DT, tag="ks0")
            nc.vector.tensor_mul(s0, src, mh0)
            s1 = pw.tile([128, HALF], MM_DT, tag="ks1")
            nc.vector.tensor_mul(s1, src, mh1)
            nc.scalar.dma_start(out=dst0, in_=s0)
            nc.scalar.dma_start(out=dst1, in_=s1)

        # ---- K projection (own tokens) + rope + staging + exchange ----
        for ug in range(2):
            accs = proj_group4(wk, ug)
            for jj in range(4):
                j = ug * 4 + jj
                t1, t2 = rope(accs[jj])
                rot = pw.tile([128, HALF], MM_DT, tag="rot")
                nc.vector.tensor_add(rot, t1, t2)
                stage_pair(rot, stageK[0, j], stageK[1, j])
        nc.gpsimd.collective_compute(
            "AllReduce", mybir.AluOpType.add, GROUPS,
            ins=[stageK], outs=[gathK])
        kT_s = kbuf.tile([128, KH, S], MM_DT, tag="k")

        # ---- V projection (transposed layout) + staging + exchange ----
        for vg in range(2):
            accs = projT_group4(wv, vg)
            for t4 in range(4):
                v_s = pw.tile([128, HALF], MM_DT, tag="qs")
                nc.scalar.copy(v_s, accs[t4])
                idx = t4 * 2 + vg
                stage_pair(v_s, stageV[0, idx], stageV[1, idx])
        nc.gpsimd.collective_compute(
            "AllReduce", mybir.AluOpType.add, GROUPS,
            ins=[stageV], outs=[gathV])
        v_g = vbuf.tile([128, 2, 8, HALF], MM_DT, tag="v")

        def v_ap(kvh, p):
            # [128 tok, 128 vd] tile of kv-head kvh at gathered position p
            return v_g[:, p // 4, (p % 4) * 2 + (kvh // 4),
                       (kvh % 4) * 128:(kvh % 4 + 1) * 128]

        # ---- Q projection + attention, interleaved per kv-group ----
        qT = qbuf.tile([128, H, HALF], MM_DT, tag="q")
        attnT = abuf.tile([128, H, HALF], MM_DT, tag="a")

        def attend(kvh, h):
            oT = ps_attn.tile([128, HALF], F32, tag="a2", name="oT")
            sum_ps = ps_attn.tile([128, HALF], F32, tag="a2", name="sum")
            for ti, t in enumerate(range(7, -1, -1)):
                cols = NCOLS[t]
                pos = TRAV[t]
                sc = ps_sc.tile([128, HALF], F32, tag="sc")
                nc.tensor.matmul(
                    sc[:, 0:cols], kT_s[:, kvh, pos * 128:(pos + 1) * 128],
                    qT[:, h, 0:cols], start=True, stop=True)
                p_t = pr.tile([128, HALF], MM_DT, tag="pr")
                nc.scalar.activation(p_t[:, 0:cols], sc[:, 0:cols],
                                     mybir.ActivationFunctionType.Exp,
                                     bias=m8, scale=SCALE)
                nc.vector.tensor_mul(p_t[:, cols - 128:cols],
                                     p_t[:, cols - 128:cols], dmask[:, t, :])
                nc.tensor.matmul(oT[:, 0:cols], v_ap(kvh, pos),
                                 p_t[:, 0:cols], start=(ti == 0),
                                 stop=(ti == 7))
                nc.tensor.matmul(sum_ps[:, 0:cols], ones, p_t[:, 0:cols],
                                 start=(ti == 0), stop=(ti == 7))
            rB2 = rb.tile([128, HALF], F32, tag="rb2")
            nc.vector.reciprocal_approx_fast(rB2, sum_ps)
            nc.vector.tensor_mul(attnT[:, h, :], oT, rB2)

        for g in range(8):
            accs = proj_group4(wq, g)
            for jj in range(4):
                t1, t2 = rope(accs[jj])
                nc.vector.tensor_add(qT[:, g * 4 + jj, :], t1, t2)
            if g == 3:
                # CC_K long done by the time the scalar queue reaches these
                for kvh in range(KH):
                    nc.scalar.dma_start(out=kT_s[:, kvh, 0:512],
                                        in_=gathK[0, kvh])
                    nc.scalar.dma_start(out=kT_s[:, kvh, 512:1024],
                                        in_=gathK[1, kvh])
            if g == 4:
                for half in range(2):
                    for idx in range(8):
                        nc.scalar.dma_start(out=v_g[:, half, idx, :],
                                            in_=gathV[half, idx])
            if g >= 5:
                for h in range(2 * (g - 5), 2 * (g - 5) + 2):
                    attend(h // 4, h)

        for h in range(6, 16):
            attend(h // 4, h)

        def out_djg(p, dst, djg):
            accs = [ps_proj.tile([128, HALF], F32, tag="acc",
                                 name=f"oacc{i}") for i in range(4)]
            for hq in range(4):
                wo_t = wbuf.tile([128, 4, HALF], MM_DT, tag="w")
                nc.sync.dma_start(
                    out=wo_t,
                    in_=wo[(p * 4 + hq) * 512:(p * 4 + hq + 1) * 512,
                           djg * 512:(djg + 1) * 512].rearrange(
                               "(t p) j -> p t j", p=128))
                for t in range(4):
                    hd = p * 16 + hq * 4 + t
                    for t4 in range(4):
                        nc.tensor.matmul(
                            accs[t4],
                            attnT[:, hd, t4 * 128:(t4 + 1) * 128],
                            wo_t[:, t, :],
                            start=(hq == 0 and t == 0),
                            stop=(hq == 3 and t == 3))
            for t4 in range(4):
                o_s = ow.tile([128, HALF], F32, tag="ow")
                if t4 % 2 == 0:
                    nc.scalar.copy(o_s, accs[t4])
                else:
                    nc.vector.tensor_copy(o_s, accs[t4])
                dq = nc.scalar if t4 % 2 == 0 else nc.sync
                dq.dma_start(
                    out=dst[t4 * 128:(t4 + 1) * 128,
                            djg * 512:(djg + 1) * 512],
                    in_=o_s)

        for djg in range(8):
            for h in (16 + 2 * djg, 17 + 2 * djg):
                attend(h // 4, h)
            out_djg(0, out0, djg)
        for djg in range(8):
            out_djg(1, out1, djg)

    nc.compile()
    return nc


def _get_compiled():
    global _compiled
    if _compiled is None:
        _compiled = _build()
    return _compiled


def _host_prep(x, freqs_cis, mask, wq, wk, wv, wo):
    """Shard + lay out inputs per core. Core 2b+h: batch b, q-row parity h."""
    del mask  # causal structure is hardcoded in the per-core mask tiles

    def ab_perm(n_heads):
        p = []
        for hh in range(n_heads):
            base = hh * HD
            p.extend(range(base, base + HD, 2))
            p.extend(range(base + 1, base + HD, 2))
        return np.asarray(p)

    wq_p = np.ascontiguousarray(np.asarray(wq)[:, ab_perm(H)]).astype(MM_NP)
    wk_p = np.ascontiguousarray(np.asarray(wk)[:, ab_perm(KH)]).astype(MM_NP)
    wv_p = np.asarray(wv).astype(MM_NP)
    wo_p = np.asarray(wo).astype(MM_NP)

    rotT = np.zeros((128, 128), dtype=MM_NP)
    rotT[np.arange(64), np.arange(64) + 64] = 1.0
    rotT[np.arange(64) + 64, np.arange(64)] = -1.0
    ones = np.ones((128, 128), dtype=MM_NP)

    x = np.asarray(x, dtype=np.float32)
    freqs = np.asarray(freqs_cis, dtype=np.float64)
    tri = (np.arange(128)[:, None] <= np.arange(128)[None, :])  # k_i <= q_j

    in_maps = []
    for c in range(N_CORES):
        b, h = divmod(c, 2)
        blocks = BLOCKS_A if h == 0 else BLOCKS_B
        perm = np.concatenate(
            [np.arange(r * 128, (r + 1) * 128) for r in blocks])
        xT_c = np.ascontiguousarray(x[b][perm].T).astype(MM_NP)
        f = freqs[perm]                        # [512, 64] angles
        cos_h = np.cos(f).T.astype(np.float32)  # [64, 512]
        sin_h = np.sin(f).T.astype(np.float32)
        cosT_c = np.ascontiguousarray(np.concatenate([cos_h, cos_h], axis=0))
        sinT_c = np.ascontiguousarray(np.concatenate([sin_h, sin_h], axis=0))
        # diag masks: A: odd t -> tri, even -> ones; B: even t -> tri, odd -> 0
        dmask_c = np.empty((128, 8, 128), dtype=MM_NP)
        for t in range(8):
            if h == 0:
                dmask_c[:, t, :] = tri if (t % 2 == 1) else 1.0
            else:
                dmask_c[:, t, :] = tri if (t % 2 == 0) else 0.0
        mh0_c = np.full((128, HALF), 1.0 - h, dtype=MM_NP)
        mh1_c = np.full((128, HALF), float(h), dtype=MM_NP)
        in_maps.append({
            "xT": xT_c, "wq": wq_p, "wk": wk_p, "wv": wv_p, "wo": wo_p,
            "cosT": cosT_c, "sinT": sinT_c, "dmask": dmask_c,
            "mh0": mh0_c, "mh1": mh1_c,
            "rotT": rotT, "ones": ones,
        })
    return in_maps


def kernel(x, freqs_cis, mask, wq, wk, wv, wo):
    nc = _get_compiled()
    in_maps = _host_prep(x, freqs_cis, mask, wq, wk, wv, wo)
    res = run_bass_kernel_spmd(nc, in_maps, list(range(N_CORES)))
    out = np.empty((B, S, D), dtype=np.float32)
    for c in range(N_CORES):
        b, h = divmod(c, 2)
        blocks = BLOCKS_A if h == 0 else BLOCKS_B
        perm = np.concatenate(
            [np.arange(r * 128, (r + 1) * 128) for r in blocks])
        out[b, perm, :] = res.results[c]["out0"] + res.results[c]["out1"]
    return out
